# revision 12
# baseline (speedup 1.0000x reference)
"""Trainium2 Bass kernel for nn_CSAB2 (cross-set attention block, 8 cores).

Sharding: zero-collective. 8 cores = 4 batches x 2 output sides (x / y).
Each core computes one full output O_x[b] or O_y[b] (1024, 1024) from its
own sequence A, the other sequence C, and the weights its side needs.

Per-core math (uniform SPMD program), mixed fp8/fp16:
  Qt  = (Wq^T A^T)            f16 matmul   (residual-sensitive)
  K1t = (Wk1^T A8^T)          fp8 DoubleRow (K=256/instr)
  K2t = (Wk2^T C8^T)          fp8 DoubleRow
  V1  = A8 Wv1, V2 = C8 Wv2   fp8 DoubleRow (token-major, + ones column)
  For attn in {own(K1,V1), oth(K2,V2)}, per head h (DH=128):
    St = K_h^T . Q8_h   zero-padded DoubleRow (stationary group1 = 0)
    P  = exp(St/32) -> fp8
    PV[i, 0:128|128] = P^T-pairs .DR. [V_h | 1]   (denominator for free)
    Z[i, h*128:] = PV * (1/denom) + Q_tok[i, h*128:]   (Pool engine)
  all LNs deferred past all exps (2 act-table loads total); LN(it) ->
  transpose -> FC(it) pipelined per token tile:
  U = Lown^T W1 + Loth^T W2 (f16 matmul); relu; LN -> out f32
g0/b0 folded into W1/W2/fcb on host. Zero biases / identity final affine
(always true for this problem's setup_inputs) are specialized away at
build time based on the actual runtime inputs; a general fallback variant
is built otherwise.
"""

import sys

sys.path.insert(0, "/opt/trn_rl_repo")

import numpy as np
import ml_dtypes

import concourse.bass as bass
import concourse.tile as tile
from concourse import bacc, mybir
from concourse.bass_utils import run_bass_kernel_spmd

N = 1024  # tokens per sequence
D = 1024  # model dim
H = 8  # heads
DH = 128  # head dim
P = 128  # partitions
NT = N // P  # 8 token tiles
FT = D // P  # 8 feature tiles
EPS = 1e-5
F8 = mybir.dt.float8e4
F16 = mybir.dt.float16
F32 = mybir.dt.float32
NF8 = ml_dtypes.float8_e4m3
SCALE = 1.0 / 32.0  # 1/sqrt(D)
DR = mybir.MatmulPerfMode.DoubleRow

_CACHED = {}


def _bcast_ap(vec_ap, cols):
    """[cols]-element DRAM vector -> [128, cols] partition-broadcast AP."""
    return bass.AP(
        tensor=vec_ap.tensor, offset=vec_ap.offset, ap=[[0, P], [1, cols]]
    )


def _stride0_pair(ap2d):
    """[128, n] AP -> [128, 2, n] with dim1 stride 0 (same data twice)."""
    return bass.AP(
        tensor=ap2d.tensor, offset=ap2d.offset,
        ap=[ap2d.ap[0], [0, 2], ap2d.ap[-1]],
    )


def _build(general):
    """Build + compile the per-core SPMD program.

    general=False specializes for zero q/k/v/fc biases and identity final
    affine (true for this problem's inputs); general=True keeps them.
    """
    nc = bacc.Bacc(None, target_bir_lowering=False, debug=False)

    dram = {}
    # transposed activations (host-side transpose): [D, N]
    dram["at"] = nc.dram_tensor("at", (D, N), F16, kind="ExternalInput")
    dram["at8"] = nc.dram_tensor("at8", (D, N), F8, kind="ExternalInput")
    dram["ct8"] = nc.dram_tensor("ct8", (D, N), F8, kind="ExternalInput")
    dram["wq"] = nc.dram_tensor("wq", (D, D), F16, kind="ExternalInput")
    for nm in ("wk1", "wk2", "wv1", "wv2"):
        dram[nm] = nc.dram_tensor(nm, (D, D), F8, kind="ExternalInput")
    for nm in ("w1", "w2"):
        dram[nm] = nc.dram_tensor(nm, (D, D), F16, kind="ExternalInput")
    if general:
        for nm in ("bq", "bk1", "bk2", "bv1", "bv2", "fcb", "g1v", "b1v"):
            dram[nm] = nc.dram_tensor(nm, (D,), F32, kind="ExternalInput")
    o_dram = nc.dram_tensor("o", (N, D), F32, kind="ExternalOutput")

    with tile.TileContext(nc) as tc:
        import contextlib

        ctx = contextlib.ExitStack()
        with ctx:
            const = ctx.enter_context(tc.tile_pool(name="const", bufs=1))
            small = ctx.enter_context(tc.tile_pool(name="small", bufs=6))
            eps_tile = const.tile([P, 1], F32, tag="eps")
            nc.vector.memset(eps_tile[:], EPS)

            bias_sb = {}
            if general:
                # per-partition bias layouts: [p, ft], element = vec[ft*128+p]
                for nm in ("bq", "bk1", "bk2"):
                    t = const.tile([P, FT], F32, tag=f"{nm}_sb")
                    nc.sync.dma_start(
                        t[:], dram[nm][:].rearrange("(t p) -> p t", p=P)
                    )
                    bias_sb[nm] = t
                for nm in ("bv1", "bv2"):
                    t = const.tile([P, D], F32, tag=f"{nm}_bc")
                    nc.sync.dma_start(t[:], _bcast_ap(dram[nm][:], D))
                    bias_sb[nm] = t

            def ln_rstd(zc_ap, nfree):
                """LN stats of zc_ap [128, nfree] -> (mean ap, rstd ap)."""
                stats = small.tile([P, 2, 6], F32, tag="ln_stats")
                for sg in range(2):
                    nc.vector.bn_stats(
                        out=stats[:, sg, :],
                        in_=zc_ap[:, sg * (nfree // 2): (sg + 1) * (nfree // 2)],
                    )
                mv = small.tile([P, 2], F32, tag="ln_mv")
                nc.vector.bn_aggr(out=mv[:], in_=stats[:])
                std = small.tile([P, 1], F32, tag="ln_std")
                nc.scalar.activation(
                    out=std[:], in_=mv[:, 1:2],
                    func=mybir.ActivationFunctionType.Sqrt,
                    bias=eps_tile[:], scale=1.0,
                )
                rstd = small.tile([P, 1], F32, tag="ln_rstd")
                nc.vector.reciprocal(out=rstd[:], in_=std[:])
                return mv[:, 0:1], rstd[:]

            # ---------------- persistent SBUF tensors ----------------
            persist = ctx.enter_context(tc.tile_pool(name="persist", bufs=1))
            lt = {
                "own": persist.tile([P, FT, N], F16, tag="lto", name="lt_own"),
                "oth": persist.tile([P, FT, N], F16, tag="ltx", name="lt_oth"),
            }
            attn_data = tc.alloc_tile_pool(name="attn_data", bufs=1)
            qt8 = attn_data.tile([P, FT, N], F8, tag="qt8", name="qt8")
            k1t = attn_data.tile([P, H, 2, N], F8, tag="k1t", name="k1t")
            k2t = attn_data.tile([P, H, 2, N], F8, tag="k2t", name="k2t")
            v1 = attn_data.tile([P, NT, H, DH + 1], F8, tag="v1", name="v1")
            v2 = attn_data.tile([P, NT, H, DH + 1], F8, tag="v2", name="v2")
            qtok_all = attn_data.tile([P, NT, D], F16, tag="qtok", name="qtok")
            zc = {
                "own": attn_data.tile([P, NT, D], F16, tag="zco", name="zc_own"),
                "oth": attn_data.tile([P, NT, D], F16, tag="zcx", name="zc_oth"),
            }
            # zero group-1 of K tiles (read by zero-padded DoubleRow QK)
            nc.gpsimd.memset(k1t[:, :, 1, :], 0.0)
            nc.gpsimd.memset(k2t[:, :, 1, :], 0.0)
            # ones columns for the PV denominators
            nc.gpsimd.memset(v1[:, :, :, DH], 1.0)
            nc.gpsimd.memset(v2[:, :, :, DH], 1.0)

            # ---------------- phase A: projections ----------------
            phase_a = tc.alloc_tile_pool(name="phase_a", bufs=1)
            at = phase_a.tile([P, FT, N], F16, tag="at", name="at_all")
            at8 = phase_a.tile([P, FT, N], F8, tag="at8", name="at8_all")
            ct8 = phase_a.tile([P, FT, N], F8, tag="ct8", name="ct8_all")
            nc.sync.dma_start(at8[:], dram["at8"][:].rearrange("(t p) n -> p t n", p=P))
            nc.sync.dma_start(ct8[:], dram["ct8"][:].rearrange("(t p) n -> p t n", p=P))
            nc.sync.dma_start(at[:], dram["at"][:].rearrange("(t p) n -> p t n", p=P))

            with (
                tc.tile_pool(name="wpan", bufs=2) as wpan,
                tc.tile_pool(name="q16pan", bufs=3) as q16pan,
                tc.tile_pool(name="psum_proj", bufs=3, space="PSUM") as psum_p,
            ):
                # K projections first (fp8 inputs land sooner: 1MB vs 2MB)
                def kproj_full(w_dram, src8, dst, bias_key):
                    for ft in range(FT):
                        wp = wpan.tile([P, FT, P], F8, tag="wp8")
                        nc.sync.dma_start(
                            wp[:],
                            w_dram[:, ft * P: (ft + 1) * P].rearrange(
                                "(dt p) f -> p dt f", p=P
                            ),
                        )
                        ps = psum_p.tile([P, N], F32, tag="ps_proj")
                        for ic in range(2):
                            for d in range(FT // 2):
                                nc.tensor.matmul(
                                    ps[:, ic * 512: (ic + 1) * 512],
                                    wp[:, 2 * d: 2 * d + 2, :],
                                    src8[:, 2 * d: 2 * d + 2, ic * 512: (ic + 1) * 512],
                                    start=(d == 0), stop=(d == FT // 2 - 1),
                                    perf_mode=DR,
                                )
                        if general:
                            nc.vector.tensor_scalar(
                                out=dst[:, ft, 0, :], in0=ps[:],
                                scalar1=bias_sb[bias_key][:, ft: ft + 1],
                                scalar2=None, op0=mybir.AluOpType.add,
                            )
                        else:
                            nc.vector.tensor_copy(dst[:, ft, 0, :], ps[:])

                kproj_full(dram["wk1"], at8, k1t, "bk1")
                kproj_full(dram["wk2"], ct8, k2t, "bk2")

                # Q projection (f16): feature-major
                for ft in range(FT):
                    wp = wpan.tile([P, FT, P], F16, tag="wp16")
                    nc.sync.dma_start(
                        wp[:],
                        dram["wq"][:, ft * P: (ft + 1) * P].rearrange(
                            "(dt p) f -> p dt f", p=P
                        ),
                    )
                    ps = psum_p.tile([P, N], F32, tag="ps_proj")
                    for ic in range(2):
                        for d in range(FT):
                            nc.tensor.matmul(
                                ps[:, ic * 512: (ic + 1) * 512],
                                wp[:, d, :],
                                at[:, d, ic * 512: (ic + 1) * 512],
                                start=(d == 0), stop=(d == FT - 1),
                            )
                    qt16 = q16pan.tile([P, N], F16, tag="qt16")
                    if general:
                        nc.vector.tensor_scalar(
                            out=qt16[:], in0=ps[:],
                            scalar1=bias_sb["bq"][:, ft: ft + 1],
                            scalar2=None, op0=mybir.AluOpType.add,
                        )
                    else:
                        nc.vector.tensor_copy(qt16[:], ps[:])
                    nc.gpsimd.tensor_copy(qt8[:, ft, :], qt16[:])
                    # token-major Q for the residual
                    nc.scalar.dma_start_transpose(
                        qtok_all[:, :, ft * P: (ft + 1) * P], qt16[:]
                    )

                # V projections (fp8 DR, token-major)
                def vproj(w_dram, src8, dst, bias_key):
                    wvs = []
                    for fc in range(2):
                        w = wpan.tile([P, FT, 512], F8, tag=f"wv8_{fc}")
                        nc.sync.dma_start(
                            w[:],
                            w_dram[:, fc * 512: (fc + 1) * 512].rearrange(
                                "(dt p) f -> p dt f", p=P
                            ),
                        )
                        wvs.append(w)
                    for j in range(NT):
                        ps = psum_p.tile([P, D], F32, tag="ps_proj")
                        for fc in range(2):
                            for d in range(FT // 2):
                                nc.tensor.matmul(
                                    ps[:, fc * 512: (fc + 1) * 512],
                                    src8[:, 2 * d: 2 * d + 2, j * P: (j + 1) * P],
                                    wvs[fc][:, 2 * d: 2 * d + 2, :],
                                    start=(d == 0), stop=(d == FT // 2 - 1),
                                    perf_mode=DR,
                                )
                        if general:
                            nc.vector.tensor_add(
                                dst[:, j, :, 0:DH],
                                ps[:].rearrange("p (h f) -> p h f", f=DH),
                                bias_sb[bias_key][:].rearrange(
                                    "p (h f) -> p h f", f=DH
                                ),
                            )
                        else:
                            nc.vector.tensor_copy(
                                dst[:, j, :, 0:DH],
                                ps[:].rearrange("p (h f) -> p h f", f=DH),
                            )

                vproj(dram["wv1"], at8, v1, "bv1")
                vproj(dram["wv2"], ct8, v2, "bv2")

            phase_a.release()

            # prefetch FC weights into the space freed by phase A
            fc_w = tc.alloc_tile_pool(name="fc_w", bufs=1)
            wps = {}
            for oc in range(2):
                for nm in ("w1", "w2"):
                    wp = fc_w.tile(
                        [P, FT, 512], F16, tag=f"wf_{nm}{oc}", name=f"wf_{nm}{oc}"
                    )
                    nc.sync.dma_start(
                        wp[:],
                        dram[nm][:, oc * 512: (oc + 1) * 512].rearrange(
                            "(dt p) f -> p dt f", p=P
                        ),
                    )
                    wps[(nm, oc)] = wp

            # ---------------- attention ----------------
            attn_pools = (
                tc.alloc_tile_pool(name="p_pool", bufs=8),
                tc.alloc_tile_pool(name="psum_s", bufs=3, space="PSUM"),
                tc.alloc_tile_pool(name="psum_pv", bufs=2, space="PSUM"),
            )
            p_pool, psum_s, psum_pv = attn_pools

            def attn_block(key, kt, v_t):
                for ic in range(2):
                    for h in range(H):
                        p_t = [
                            p_pool.tile([P, 2, 512], F8, tag="p_t", name="p_t")
                            for _ in range(NT // 2)
                        ]
                        qmov = _stride0_pair(qt8[:, h, ic * 512: (ic + 1) * 512])
                        for jp in range(NT // 2):
                            sps = psum_s.tile([P, 2, 512], F32, tag="sps")
                            for half in range(2):
                                jt = 2 * jp + half
                                nc.tensor.matmul(
                                    sps[:, half, :],
                                    kt[:, h, :, jt * P: (jt + 1) * P],
                                    qmov,
                                    start=True, stop=True,
                                    perf_mode=DR,
                                )
                            nc.scalar.activation(
                                out=p_t[jp][:], in_=sps[:],
                                func=mybir.ActivationFunctionType.Exp,
                                scale=SCALE,
                            )
                        for ip in range(2):
                            pvp = psum_pv.tile([P, 2, DH + 1], F32, tag="pvp")
                            for half in range(2):
                                il = 2 * ip + half
                                for jp in range(NT // 2):
                                    nc.tensor.matmul(
                                        pvp[:, half, :],
                                        p_t[jp][:, :, il * P: (il + 1) * P],
                                        v_t[:, 2 * jp: 2 * jp + 2, h, :],
                                        start=(jp == 0), stop=(jp == NT // 2 - 1),
                                        perf_mode=DR,
                                    )
                            rcp = small.tile([P, 2], F32, tag="rcp")
                            nc.vector.reciprocal(rcp[:], pvp[:, :, DH])
                            for half in range(2):
                                il = 2 * ip + half
                                it = ic * 4 + il
                                nc.vector.scalar_tensor_tensor(
                                    out=zc[key][:, it, h * DH: (h + 1) * DH],
                                    in0=pvp[:, half, 0:DH],
                                    scalar=rcp[:, half: half + 1],
                                    in1=qtok_all[:, it, h * DH: (h + 1) * DH],
                                    op0=mybir.AluOpType.mult,
                                    op1=mybir.AluOpType.add,
                                )

            attn_block("own", k1t, v1)
            attn_block("oth", k2t, v2)

            for pool in reversed(attn_pools):
                pool.release()

            # ---------------- LN + FC pipelined per token tile ----------------
            if general:
                fc_const = ctx.enter_context(tc.tile_pool(name="fc_const", bufs=1))
                fcb_bc = fc_const.tile([P, D], F32, tag="fcb_bc")
                nc.sync.dma_start(fcb_bc[:], _bcast_ap(dram["fcb"][:], D))
                g1_bc = fc_const.tile([P, D], F32, tag="g1_bc")
                nc.sync.dma_start(g1_bc[:], _bcast_ap(dram["g1v"][:], D))
                b1_bc = fc_const.tile([P, D], F32, tag="b1_bc")
                nc.sync.dma_start(b1_bc[:], _bcast_ap(dram["b1v"][:], D))

            with (
                tc.tile_pool(name="ltok_pool", bufs=4) as ltok_pool,
                tc.tile_pool(name="u_pool", bufs=3) as u_pool,
                tc.tile_pool(name="psum_fc", bufs=3, space="PSUM") as psum_fc,
                tc.tile_pool(name="out_pool", bufs=3) as out_pool,
            ):
                for it in range(NT):
                    # LN both attention outputs for this token tile
                    for key in ("own", "oth"):
                        mean, rstd = ln_rstd(zc[key][:, it, :], D)
                        ltok = ltok_pool.tile([P, D], F16, tag="ltok")
                        nc.gpsimd.tensor_scalar(
                            out=ltok[:], in0=zc[key][:, it, :],
                            scalar1=mean, scalar2=rstd,
                            op0=mybir.AluOpType.subtract,
                            op1=mybir.AluOpType.mult,
                        )
                        nc.scalar.dma_start_transpose(
                            lt[key][:, :, it * P: (it + 1) * P], ltok[:]
                        )
                    # FC for this token tile
                    ut = u_pool.tile([P, D], F16, tag="ut", name="ut")
                    for oc in range(2):
                        fps = psum_fc.tile([P, 512], F32, tag="fps")
                        for kt in range(FT):
                            nc.tensor.matmul(
                                fps[:],
                                lt["own"][:, kt, it * P: (it + 1) * P],
                                wps[("w1", oc)][:, kt, :],
                                start=(kt == 0), stop=False,
                            )
                        for kt in range(FT):
                            nc.tensor.matmul(
                                fps[:],
                                lt["oth"][:, kt, it * P: (it + 1) * P],
                                wps[("w2", oc)][:, kt, :],
                                start=False, stop=(kt == FT - 1),
                            )
                        if general:
                            nc.vector.scalar_tensor_tensor(
                                out=ut[:, oc * 512: (oc + 1) * 512],
                                in0=fps[:], scalar=1.0,
                                in1=fcb_bc[:, oc * 512: (oc + 1) * 512],
                                op0=mybir.AluOpType.mult,
                                op1=mybir.AluOpType.add,
                            )
                        else:
                            nc.vector.tensor_scalar(
                                out=ut[:, oc * 512: (oc + 1) * 512],
                                in0=fps[:], scalar1=0.0, scalar2=None,
                                op0=mybir.AluOpType.max,
                            )
                    if general:
                        nc.gpsimd.tensor_scalar(
                            out=ut[:], in0=ut[:], scalar1=0.0, scalar2=None,
                            op0=mybir.AluOpType.max,
                        )
                    mean, rstd = ln_rstd(ut[:], D)
                    ot = out_pool.tile([P, D], F32, tag="ot")
                    nc.gpsimd.tensor_scalar(
                        out=ot[:], in0=ut[:], scalar1=mean, scalar2=rstd,
                        op0=mybir.AluOpType.subtract,
                        op1=mybir.AluOpType.mult,
                    )
                    if general:
                        nc.vector.tensor_mul(ot[:], ot[:], g1_bc[:])
                        nc.vector.tensor_add(ot[:], ot[:], b1_bc[:])
                    nc.sync.dma_start(o_dram[it * P: (it + 1) * P, :], ot[:])

            fc_w.release()
            attn_data.release()

    nc.compile()
    return nc


def build_in_maps(X, Y, Wqx, bqx, Wkx, bkx, Wvx, bvx, Wqy, bqy, Wky, bky,
                  Wvy, bvy, WX, bX, WY, bY, g0, b0, g1, b1):
    f = lambda t: np.asarray(t, dtype=np.float32)
    h16 = lambda t: np.ascontiguousarray(f(t).astype(np.float16))
    h8 = lambda t: np.ascontiguousarray(f(t).astype(np.float16).astype(NF8))
    X, Y = f(X), f(Y)
    g0d, b0d = f(g0).astype(np.float64), f(b0).astype(np.float64)

    general = not (
        all(np.allclose(f(b), 0.0) for b in (bqx, bkx, bvx, bqy, bky, bvy,
                                             bX, bY, b0, b1))
        and np.allclose(f(g1), 1.0)
    )

    sides = {}
    for side, W, bo in (("x", f(WX), f(bX)), ("y", f(WY), f(bY))):
        Wtop = W[:D].astype(np.float64)
        Wbot = W[D:].astype(np.float64)
        fcb = (b0d @ Wtop + b0d @ Wbot + bo.astype(np.float64)).astype(np.float32)
        w_top_folded = (g0d[:, None] * Wtop).astype(np.float32)
        w_bot_folded = (g0d[:, None] * Wbot).astype(np.float32)
        if side == "x":
            w_own, w_oth = w_top_folded, w_bot_folded  # [O_xx, O_xy]
        else:
            w_own, w_oth = w_bot_folded, w_top_folded  # [O_yx, O_yy]
        sides[side] = dict(w1=h16(w_own), w2=h16(w_oth), fcb=fcb)

    wx = dict(wq=h16(Wqx), bq=f(bqx), wk=h8(Wkx), bk=f(bkx), wv=h8(Wvx),
              bv=f(bvx))
    wy = dict(wq=h16(Wqy), bq=f(bqy), wk=h8(Wky), bk=f(bky), wv=h8(Wvy),
              bv=f(bvy))

    seq_t = {}
    for nm, S in (("x", X), ("y", Y)):
        for b in range(S.shape[0]):
            t16 = np.ascontiguousarray(S[b].T.astype(np.float16))
            seq_t[(nm, b)] = (t16, np.ascontiguousarray(t16.astype(NF8)))

    in_maps = []
    for core in range(8):
        b = core // 2
        side = "x" if core % 2 == 0 else "y"
        own, oth = (wx, wy) if side == "x" else (wy, wx)
        a_t = seq_t[(side, b)]
        c_t = seq_t[("y" if side == "x" else "x", b)]
        m = {
            "at": a_t[0], "at8": a_t[1], "ct8": c_t[1],
            "wq": own["wq"],
            "wk1": own["wk"], "wv1": own["wv"],
            "wk2": oth["wk"], "wv2": oth["wv"],
            "w1": sides[side]["w1"], "w2": sides[side]["w2"],
        }
        if general:
            m.update({
                "bq": own["bq"], "bk1": own["bk"], "bv1": own["bv"],
                "bk2": oth["bk"], "bv2": oth["bv"],
                "fcb": sides[side]["fcb"], "g1v": f(g1), "b1v": f(b1),
            })
        in_maps.append(m)
    return in_maps, general


def kernel(**inputs):
    in_maps, general = build_in_maps(**inputs)
    key = ("nc", general)
    if key not in _CACHED:
        _CACHED[key] = _build(general)
    nc = _CACHED[key]
    _CACHED["nc"] = nc  # test.py compatibility

    res = run_bass_kernel_spmd(nc, in_maps, list(range(8)))
    _CACHED["last_result"] = res

    B = np.asarray(inputs["X"]).shape[0]
    O_x = np.stack([res.results[2 * b]["o"] for b in range(B)])
    O_y = np.stack([res.results[2 * b + 1]["o"] for b in range(B)])
    return O_x, O_y


# revision 23
# speedup vs baseline: 1.4772x; 1.4772x over previous
"""Trainium2 Bass kernel for nn_CSAB2 (cross-set attention block, 8 cores).

Sharding: zero-collective. 8 cores = 4 batches x 2 output sides (x / y).
Each core computes one full output O_x[b] or O_y[b] (1024, 1024).

Mixed fp8/fp16 design, software-pipelined around the Act-engine exp spine
(16.8M softmax exps/core are the irreducible Act-bound phase):

  block1 (own ic0):  per head h: K1(h) fp8-DR proj, Q(h) f16 proj,
                     V1(h) fp8-DR proj, then attention(own, ic0, h)
  block2 (oth ic0):  per head: K2(h), V2(h) projections + attention
  LN+transpose of token tiles 0-3 (both attns)
  block3 (own ic1):  attention + interleaved FC matmul half-groups
  block4 (oth ic1):  attention + FC + LN(own, tiles 4-7)
  tail:              LN(oth, 4-7), FC(4-7), relu, final LN, store

Attention per (ic, h): St = K_h^T.DR.[Q8|0] (stride-0 stationary pair,
moving zero row in qzz), P = exp(St/32) -> fp8, PV via fp8-DR over
paired j-tiles with an appended ones column (denominator for free),
Z = PV*(1/den) + Qtok (DVE), LN deferred.

LN rstd = Exp(-0.5*Ln(var+eps)) so the Act engine keeps one table
(natural_log_exp_and_others) loaded for the entire kernel - no act-table
swaps between softmax exps and LN.

g0/b0 folded into W1/W2/fcb on host. Zero biases / identity final affine
(always true for this problem's setup_inputs) are specialized away at
build time from the actual runtime inputs; a general fallback variant is
built otherwise.
"""

import sys

sys.path.insert(0, "/opt/trn_rl_repo")

import numpy as np
import ml_dtypes

import concourse.bass as bass
import concourse.tile as tile
from concourse import bacc, mybir
from concourse.bass_utils import run_bass_kernel_spmd

N = 1024  # tokens per sequence
D = 1024  # model dim
H = 8  # heads
DH = 128  # head dim
P = 128  # partitions
NT = N // P  # 8 token tiles
FT = D // P  # 8 feature tiles
EPS = 1e-5
F8 = mybir.dt.float8e4
F16 = mybir.dt.float16
F32 = mybir.dt.float32
NF8 = ml_dtypes.float8_e4m3
SCALE = 1.0 / 32.0  # 1/sqrt(D)
DR = mybir.MatmulPerfMode.DoubleRow

_CACHED = {}


def _bcast_ap(vec_ap, cols):
    """[cols]-element DRAM vector -> [128, cols] partition-broadcast AP."""
    return bass.AP(
        tensor=vec_ap.tensor, offset=vec_ap.offset, ap=[[0, P], [1, cols]]
    )


def _pair(ap2d, stride):
    """[128, n] AP -> [128, 2, n] inserting a dim-1 of the given stride."""
    return bass.AP(
        tensor=ap2d.tensor, offset=ap2d.offset,
        ap=[ap2d.ap[0], [stride, 2], ap2d.ap[-1]],
    )


def _build(general):
    nc = bacc.Bacc(None, target_bir_lowering=False, debug=False)

    dram = {}
    # transposed activations (host-side transpose): [D, N]
    dram["at"] = nc.dram_tensor("at", (D, N), F16, kind="ExternalInput")
    dram["at8"] = nc.dram_tensor("at8", (D, N), F8, kind="ExternalInput")
    dram["ct8"] = nc.dram_tensor("ct8", (D, N), F8, kind="ExternalInput")
    dram["wq"] = nc.dram_tensor("wq", (D, D), F16, kind="ExternalInput")
    for nm in ("wk1", "wk2", "wv1", "wv2"):
        dram[nm] = nc.dram_tensor(nm, (D, D), F8, kind="ExternalInput")
    for nm in ("w1", "w2"):
        dram[nm] = nc.dram_tensor(nm, (D, D), F16, kind="ExternalInput")
    if general:
        for nm in ("bq", "bk1", "bk2", "bv1", "bv2", "fcb", "g1v", "b1v"):
            dram[nm] = nc.dram_tensor(nm, (D,), F32, kind="ExternalInput")
    o_dram = nc.dram_tensor("o", (N, D), F32, kind="ExternalOutput")

    with tile.TileContext(nc) as tc:
        import contextlib

        ctx = contextlib.ExitStack()
        with ctx:
            const = ctx.enter_context(tc.tile_pool(name="const", bufs=1))
            small = ctx.enter_context(tc.tile_pool(name="small", bufs=8))
            eps_tile = const.tile([P, 1], F32, tag="eps")
            nc.vector.memset(eps_tile[:], EPS)

            bias_sb = {}
            if general:
                for nm in ("bq", "bk1", "bk2"):
                    t = const.tile([P, FT], F32, tag=f"{nm}_sb")
                    nc.sync.dma_start(
                        t[:], dram[nm][:].rearrange("(t p) -> p t", p=P)
                    )
                    bias_sb[nm] = t
                for nm in ("bv1", "bv2", "fcb", "g1v", "b1v"):
                    t = const.tile([P, D], F32, tag=f"{nm}_bc")
                    nc.sync.dma_start(t[:], _bcast_ap(dram[nm][:], D))
                    bias_sb[nm] = t

            # ---------------- persistent SBUF ----------------
            persist = ctx.enter_context(tc.tile_pool(name="persist", bufs=1))
            lt = {
                "own": persist.tile([P, FT, N], F16, tag="lto", name="lt_own"),
                "oth": persist.tile([P, FT, N], F16, tag="ltx", name="lt_oth"),
            }
            attn_data = tc.alloc_tile_pool(name="attn_data", bufs=1)
            ctx.callback(attn_data.release)
            # qzz row FT is all zeros: the moving zero group for DR QK
            qzz = attn_data.tile([P, FT + 1, N], F8, tag="qzz", name="qzz")
            k1t = attn_data.tile([P, H, N], F8, tag="k1t", name="k1t")
            k2t = attn_data.tile([P, H, N], F8, tag="k2t", name="k2t")
            v1 = attn_data.tile([P, NT, H, DH + 1], F8, tag="v1", name="v1")
            v2 = attn_data.tile([P, NT, H, DH + 1], F8, tag="v2", name="v2")
            qtok = attn_data.tile([P, NT, D], F16, tag="qtok", name="qtok")
            zc = {
                "own": attn_data.tile([P, NT, D], F16, tag="zco", name="zc_own"),
                "oth": attn_data.tile([P, NT, D], F16, tag="zcx", name="zc_oth"),
            }
            nc.gpsimd.memset(qzz[:, FT, :], 0.0)
            nc.gpsimd.memset(v1[:, :, :, DH], 1.0)
            nc.gpsimd.memset(v2[:, :, :, DH], 1.0)

            # FC weights: allocated early (LIFO), DMA'd after block1 panels
            fc_w = tc.alloc_tile_pool(name="fc_w", bufs=1)
            ctx.callback(fc_w.release)
            wps = {}
            for oc in range(2):
                for nm in ("w1", "w2"):
                    wps[(nm, oc)] = fc_w.tile(
                        [P, FT, 512], F16, tag=f"wf_{nm}{oc}", name=f"wf_{nm}{oc}"
                    )

            # phase-A inputs (released after block1; right side so the
            # release is independent of the left-side pool stack)
            pool_a = tc.alloc_tile_pool(name="pool_a", bufs=1, side="right")
            at = pool_a.tile([P, FT, N], F16, tag="at", name="at_all")
            at8 = pool_a.tile([P, FT, N], F8, tag="at8", name="at8_all")
            nc.sync.dma_start(at8[:], dram["at8"][:].rearrange("(t p) n -> p t n", p=P))
            nc.gpsimd.dma_start(at[:], dram["at"][:].rearrange("(t p) n -> p t n", p=P))

            # rings
            wq_pan = ctx.enter_context(tc.tile_pool(name="wq_pan", bufs=2))
            wk_pan = ctx.enter_context(tc.tile_pool(name="wk_pan", bufs=2))
            wv_pan = ctx.enter_context(tc.tile_pool(name="wv_pan", bufs=2))
            q16_pan = ctx.enter_context(tc.tile_pool(name="q16_pan", bufs=2))
            p_pool = ctx.enter_context(tc.tile_pool(name="p_pool", bufs=4))
            ltok_pool = ctx.enter_context(tc.tile_pool(name="ltok_pool", bufs=3))
            u_pool = ctx.enter_context(tc.tile_pool(name="u_pool", bufs=2))
            out_pool = ctx.enter_context(tc.tile_pool(name="out_pool", bufs=1))
            psum_s = tc.alloc_tile_pool(name="psum_s", bufs=2, space="PSUM")
            psum_pv = tc.alloc_tile_pool(name="psum_pv", bufs=2, space="PSUM")
            psum_pr = tc.alloc_tile_pool(name="psum_pr", bufs=2, space="PSUM")

            def ln_rstd(src_ap):
                """LN stats of src_ap [128, D] -> (mean ap, rstd ap).
                rstd = Exp(-0.5 * Ln(var + eps)): keeps the Act engine on the
                natural_log_exp table (no swaps against softmax exps)."""
                stats = small.tile([P, 2, 6], F32, tag="ln_stats")
                for sg in range(2):
                    nc.vector.bn_stats(
                        out=stats[:, sg, :], in_=src_ap[:, sg * 512: (sg + 1) * 512]
                    )
                mv = small.tile([P, 2], F32, tag="ln_mv")
                nc.vector.bn_aggr(out=mv[:], in_=stats[:])
                lnv = small.tile([P, 1], F32, tag="ln_lnv")
                nc.scalar.activation(
                    out=lnv[:], in_=mv[:, 1:2],
                    func=mybir.ActivationFunctionType.Ln,
                    bias=eps_tile[:], scale=1.0,
                )
                rstd = small.tile([P, 1], F32, tag="ln_rstd")
                nc.scalar.activation(
                    out=rstd[:], in_=lnv[:],
                    func=mybir.ActivationFunctionType.Exp,
                    scale=-0.5,
                )
                return mv[:, 0:1], rstd[:]

            def kproj(h, w_dram, src8, dst, bias_key):
                """dst[:, h, :] = (W[:, h-feats]^T src8)  (fp8 DoubleRow)."""
                wp = wk_pan.tile([P, FT, P], F8, tag="wp8")
                nc.sync.dma_start(
                    wp[:],
                    w_dram[:, h * P: (h + 1) * P].rearrange("(dt p) f -> p dt f", p=P),
                )
                for ic in range(2):
                    ps = psum_pr.tile([P, 512], F32, tag="psA")
                    for d in range(FT // 2):
                        nc.tensor.matmul(
                            ps[:],
                            wp[:, 2 * d: 2 * d + 2, :],
                            src8[:, 2 * d: 2 * d + 2, ic * 512: (ic + 1) * 512],
                            start=(d == 0), stop=(d == FT // 2 - 1),
                            perf_mode=DR,
                        )
                    out = dst[:, h, ic * 512: (ic + 1) * 512]
                    if general:
                        nc.vector.tensor_scalar(
                            out=out, in0=ps[:],
                            scalar1=bias_sb[bias_key][:, h: h + 1],
                            scalar2=None, op0=mybir.AluOpType.add,
                        )
                    else:
                        nc.vector.tensor_copy(out, ps[:])

            def qproj(h):
                """qzz[:, h, :], qtok[:, :, h-feats] = A @ Wq[:, h-feats] (f16)."""
                wp = wq_pan.tile([P, FT, P], F16, tag="wp16")
                nc.sync.dma_start(
                    wp[:],
                    dram["wq"][:, h * P: (h + 1) * P].rearrange(
                        "(dt p) f -> p dt f", p=P
                    ),
                )
                for ic in range(2):
                    ps = psum_pr.tile([P, 512], F32, tag="psA")
                    for d in range(FT):
                        nc.tensor.matmul(
                            ps[:],
                            wp[:, d, :],
                            at[:, d, ic * 512: (ic + 1) * 512],
                            start=(d == 0), stop=(d == FT - 1),
                        )
                    q16 = q16_pan.tile([P, 512], F16, tag="q16")
                    if general:
                        nc.vector.tensor_scalar(
                            out=q16[:], in0=ps[:],
                            scalar1=bias_sb["bq"][:, h: h + 1],
                            scalar2=None, op0=mybir.AluOpType.add,
                        )
                    else:
                        nc.vector.tensor_copy(q16[:], ps[:])
                    nc.gpsimd.tensor_copy(qzz[:, h, ic * 512: (ic + 1) * 512], q16[:])
                    nc.sync.dma_start_transpose(
                        qtok[:, ic * 4: (ic + 1) * 4, h * P: (h + 1) * P], q16[:]
                    )

            def vproj(h, w_dram, src8, dst, bias_key):
                """dst[:, :, h, 0:DH] = src8^T @ W[:, h-feats]  (fp8 DR)."""
                wp = wv_pan.tile([P, FT, P], F8, tag="wv8")
                nc.sync.dma_start(
                    wp[:],
                    w_dram[:, h * P: (h + 1) * P].rearrange("(dt p) f -> p dt f", p=P),
                )
                for jh in range(2):
                    ps0 = psum_pr.tile([P, 512], F32, tag="psA")
                    ps = ps0[:].rearrange("p (j f) -> p j f", f=P)
                    for j in range(4):
                        for d in range(FT // 2):
                            nc.tensor.matmul(
                                ps[:, j, :],
                                src8[:, 2 * d: 2 * d + 2,
                                     (4 * jh + j) * P: (4 * jh + j + 1) * P],
                                wp[:, 2 * d: 2 * d + 2, :],
                                start=(d == 0), stop=(d == FT // 2 - 1),
                                perf_mode=DR,
                            )
                    out = dst[:, 4 * jh: 4 * jh + 4, h, 0:DH]
                    if general:
                        bc = bias_sb[bias_key][:, h * DH: (h + 1) * DH]
                        bc4 = bass.AP(tensor=bc.tensor, offset=bc.offset,
                                      ap=[bc.ap[0], [0, 4], bc.ap[-1]])
                        nc.vector.tensor_add(out, ps, bc4)
                    else:
                        nc.vector.tensor_copy(out, ps)

            def attn(key, kt, v_t, ic, h):
                """One head of one attention for one query half."""
                # moving operand: Q8 row h paired with the zero row FT
                base = qzz[:, h, ic * 512: (ic + 1) * 512]
                qmov = _pair(base, (FT - h) * N)
                p_t = []
                for jp in range(4):
                    sps = psum_s.tile([P, 2, 512], F32, tag="sps")
                    for half in range(2):
                        jt = 2 * jp + half
                        nc.tensor.matmul(
                            sps[:, half, :],
                            _pair(kt[:, h, jt * P: (jt + 1) * P], 0),
                            qmov,
                            start=True, stop=True,
                            perf_mode=DR,
                        )
                    pt = p_pool.tile([P, 2, 512], F8, tag="p_t", name="p_t")
                    nc.scalar.activation(
                        out=pt[:], in_=sps[:],
                        func=mybir.ActivationFunctionType.Exp,
                        scale=SCALE,
                    )
                    p_t.append(pt)
                for ip in range(2):
                    pvp = psum_pv.tile([P, 2, DH + 1], F32, tag="pvp")
                    for half in range(2):
                        il = 2 * ip + half
                        for jp in range(4):
                            nc.tensor.matmul(
                                pvp[:, half, :],
                                p_t[jp][:, :, il * P: (il + 1) * P],
                                v_t[:, 2 * jp: 2 * jp + 2, h, :],
                                start=(jp == 0), stop=(jp == 3),
                                perf_mode=DR,
                            )
                    rcp = small.tile([P, 2], F32, tag="rcp")
                    nc.vector.reciprocal(rcp[:], pvp[:, :, DH])
                    for half in range(2):
                        il = 2 * ip + half
                        it = ic * 4 + il
                        nc.vector.scalar_tensor_tensor(
                            out=zc[key][:, it, h * DH: (h + 1) * DH],
                            in0=pvp[:, half, 0:DH],
                            scalar=rcp[:, half: half + 1],
                            in1=qtok[:, it, h * DH: (h + 1) * DH],
                            op0=mybir.AluOpType.mult,
                            op1=mybir.AluOpType.add,
                        )

            def emit_ln(key, it):
                """LN zc[key][:, it, :] -> lt[key] feature-major (transpose)."""
                mean, rstd = ln_rstd(zc[key][:, it, :])
                ltok = ltok_pool.tile([P, D], F16, tag="ltok")
                nc.gpsimd.tensor_scalar(
                    out=ltok[:], in0=zc[key][:, it, :],
                    scalar1=mean, scalar2=rstd,
                    op0=mybir.AluOpType.subtract, op1=mybir.AluOpType.mult,
                )
                nc.sync.dma_start_transpose(
                    lt[key][:, :, it * P: (it + 1) * P], ltok[:]
                )

            # ---------------- block 1: own-side projections + attn ic0 ------
            for h in range(H):
                kproj(h, dram["wk1"], at8, k1t, "bk1")
                qproj(h)
                vproj(h, dram["wv1"], at8, v1, "bv1")
                attn("own", k1t, v1, 0, h)

            # release A-side inputs; C side reuses the space
            pool_a.release()
            pool_c = tc.alloc_tile_pool(name="pool_c", bufs=1, side="right")
            ct8 = pool_c.tile([P, FT, N], F8, tag="ct8", name="ct8_all")
            nc.sync.dma_start(ct8[:], dram["ct8"][:].rearrange("(t p) n -> p t n", p=P))
            # FC weights stream in on the gpsimd (SWDGE) queue
            for oc in range(2):
                for nm in ("w1", "w2"):
                    nc.gpsimd.dma_start(
                        wps[(nm, oc)][:],
                        dram[nm][:, oc * 512: (oc + 1) * 512].rearrange(
                            "(dt p) f -> p dt f", p=P
                        ),
                    )

            # ---------------- block 2: oth-side projections + attn ic0 ------
            for h in range(H):
                kproj(h, dram["wk2"], ct8, k2t, "bk2")
                vproj(h, dram["wv2"], ct8, v2, "bv2")
                attn("oth", k2t, v2, 0, h)

            # proj psum banks -> FC psum
            psum_pr.release()
            psum_fc = tc.alloc_tile_pool(name="psum_fc", bufs=2, space="PSUM")

            # LN token tiles 0-3 (both attns) while block 3 runs
            for it in range(4):
                emit_ln("own", it)
                emit_ln("oth", it)

            # --------- FC machinery: 16 matmuls per (it, oc) group ----------
            fc_state = {}

            def fc_half(idx):
                """Half-group idx: 8 of the 16 matmuls of group g = idx//2."""
                g = idx // 2
                it, oc = g // 2, g % 2
                key, wnm = (("own", "w1") if idx % 2 == 0 else ("oth", "w2"))
                if idx % 2 == 0:
                    fc_state[g] = psum_fc.tile([P, 512], F32, tag="fps", name="fps")
                    if oc == 0:
                        fc_state[("u", it)] = u_pool.tile(
                            [P, D], F16, tag="ut", name="ut"
                        )
                fps = fc_state[g]
                for kt_i in range(FT):
                    nc.tensor.matmul(
                        fps[:],
                        lt[key][:, kt_i, it * P: (it + 1) * P],
                        wps[(wnm, oc)][:, kt_i, :],
                        start=(idx % 2 == 0 and kt_i == 0),
                        stop=(idx % 2 == 1 and kt_i == FT - 1),
                    )
                if idx % 2 == 1:
                    ut = fc_state[("u", it)]
                    if general:
                        nc.vector.scalar_tensor_tensor(
                            out=ut[:, oc * 512: (oc + 1) * 512],
                            in0=fps[:], scalar=1.0,
                            in1=bias_sb["fcb"][:, oc * 512: (oc + 1) * 512],
                            op0=mybir.AluOpType.mult, op1=mybir.AluOpType.add,
                        )
                    else:
                        nc.vector.tensor_scalar(
                            out=ut[:, oc * 512: (oc + 1) * 512],
                            in0=fps[:], scalar1=0.0, scalar2=None,
                            op0=mybir.AluOpType.max,
                        )
                    if oc == 1:
                        finish_token(it)

            def finish_token(it):
                """relu'd ut complete: final LN + store."""
                ut = fc_state.pop(("u", it))
                if general:
                    nc.gpsimd.tensor_scalar(
                        out=ut[:], in0=ut[:], scalar1=0.0, scalar2=None,
                        op0=mybir.AluOpType.max,
                    )
                mean, rstd = ln_rstd(ut[:])
                ot = out_pool.tile([P, D], F32, tag="ot")
                nc.gpsimd.tensor_scalar(
                    out=ot[:], in0=ut[:], scalar1=mean, scalar2=rstd,
                    op0=mybir.AluOpType.subtract, op1=mybir.AluOpType.mult,
                )
                if general:
                    nc.vector.tensor_mul(ot[:], ot[:], bias_sb["g1v"][:])
                    nc.vector.tensor_add(ot[:], ot[:], bias_sb["b1v"][:])
                nc.sync.dma_start(o_dram[it * P: (it + 1) * P, :], ot[:])

            # ---------------- block 3: own attn ic1 + FC(0-5 halves) --------
            for h in range(H):
                attn("own", k1t, v1, 1, h)
                if h >= 2:
                    fc_half(h - 2)  # halves 0..5

            # ---------------- block 4: oth attn ic1 + FC + LN(own,4-7) ------
            for h in range(H):
                attn("oth", k2t, v2, 1, h)
                fc_half(6 + h)  # halves 6..13
                if h >= 4:
                    emit_ln("own", h)  # token tiles 4..7

            # ---------------- tail ----------------
            fc_half(14)
            fc_half(15)
            for it in range(4, NT):
                emit_ln("oth", it)
            for idx in range(16, 32):
                fc_half(idx)

            psum_fc.release()
            psum_pv.release()
            psum_s.release()
            pool_c.release()

    nc.compile()
    return nc


def build_in_maps(X, Y, Wqx, bqx, Wkx, bkx, Wvx, bvx, Wqy, bqy, Wky, bky,
                  Wvy, bvy, WX, bX, WY, bY, g0, b0, g1, b1):
    f = lambda t: np.asarray(t, dtype=np.float32)
    h16 = lambda t: np.ascontiguousarray(f(t).astype(np.float16))
    h8 = lambda t: np.ascontiguousarray(f(t).astype(np.float16).astype(NF8))
    X, Y = f(X), f(Y)
    g0d, b0d = f(g0).astype(np.float64), f(b0).astype(np.float64)

    general = not (
        all(np.allclose(f(b), 0.0) for b in (bqx, bkx, bvx, bqy, bky, bvy,
                                             bX, bY, b0, b1))
        and np.allclose(f(g1), 1.0)
    )

    sides = {}
    for side, W, bo in (("x", f(WX), f(bX)), ("y", f(WY), f(bY))):
        Wtop = W[:D].astype(np.float64)
        Wbot = W[D:].astype(np.float64)
        fcb = (b0d @ Wtop + b0d @ Wbot + bo.astype(np.float64)).astype(np.float32)
        w_top_folded = (g0d[:, None] * Wtop).astype(np.float32)
        w_bot_folded = (g0d[:, None] * Wbot).astype(np.float32)
        if side == "x":
            w_own, w_oth = w_top_folded, w_bot_folded  # [O_xx, O_xy]
        else:
            w_own, w_oth = w_bot_folded, w_top_folded  # [O_yx, O_yy]
        sides[side] = dict(w1=h16(w_own), w2=h16(w_oth), fcb=fcb)

    wx = dict(wq=h16(Wqx), bq=f(bqx), wk=h8(Wkx), bk=f(bkx), wv=h8(Wvx),
              bv=f(bvx))
    wy = dict(wq=h16(Wqy), bq=f(bqy), wk=h8(Wky), bk=f(bky), wv=h8(Wvy),
              bv=f(bvy))

    seq_t = {}
    for nm, S in (("x", X), ("y", Y)):
        for b in range(S.shape[0]):
            t16 = np.ascontiguousarray(S[b].T.astype(np.float16))
            seq_t[(nm, b)] = (t16, np.ascontiguousarray(t16.astype(NF8)))

    in_maps = []
    for core in range(8):
        b = core // 2
        side = "x" if core % 2 == 0 else "y"
        own, oth = (wx, wy) if side == "x" else (wy, wx)
        a_t = seq_t[(side, b)]
        c_t = seq_t[("y" if side == "x" else "x", b)]
        m = {
            "at": a_t[0], "at8": a_t[1], "ct8": c_t[1],
            "wq": own["wq"],
            "wk1": own["wk"], "wv1": own["wv"],
            "wk2": oth["wk"], "wv2": oth["wv"],
            "w1": sides[side]["w1"], "w2": sides[side]["w2"],
        }
        if general:
            m.update({
                "bq": own["bq"], "bk1": own["bk"], "bv1": own["bv"],
                "bk2": oth["bk"], "bv2": oth["bv"],
                "fcb": sides[side]["fcb"], "g1v": f(g1), "b1v": f(b1),
            })
        in_maps.append(m)
    return in_maps, general


def kernel(**inputs):
    in_maps, general = build_in_maps(**inputs)
    key = ("nc", general)
    if key not in _CACHED:
        _CACHED[key] = _build(general)
    nc = _CACHED[key]
    _CACHED["nc"] = nc  # test.py compatibility

    res = run_bass_kernel_spmd(nc, in_maps, list(range(8)))
    _CACHED["last_result"] = res

    B = np.asarray(inputs["X"]).shape[0]
    O_x = np.stack([res.results[2 * b]["o"] for b in range(B)])
    O_y = np.stack([res.results[2 * b + 1]["o"] for b in range(B)])
    return O_x, O_y


# revision 38
# speedup vs baseline: 1.6812x; 1.1381x over previous
"""Trainium2 Bass kernel for nn_CSAB2 (cross-set attention block, 8 cores).

Sharding: zero-collective. 8 cores = 4 batches x 2 output sides (x / y).
Each core computes one full output O_x[b] or O_y[b] (1024, 1024).

Mixed fp8/fp16 design, software-pipelined around the Act-engine exp spine
(16.8M softmax exps/core are the irreducible Act-bound phase):

  block1 (own ic0):  per head h: K1(h) fp8-DR proj, Q(h) f16 proj,
                     V1(h) fp8-DR proj, then attention(own, ic0, h)
  block2 (oth ic0):  per head: K2(h), V2(h) projections + attention
  LN+transpose of token tiles 0-3 (both attns)
  block3 (own ic1):  attention + interleaved FC matmul half-groups
  block4 (oth ic1):  attention + FC + LN(own, tiles 4-7)
  tail:              LN(oth, 4-7), FC(4-7), relu, final LN, store

Attention per (ic, h): St = K_h^T.DR.[Q8|0] (stride-0 stationary pair,
moving zero row in qzz), P = exp(St/32) -> fp8, PV via fp8-DR over
paired j-tiles with an appended ones column (denominator for free),
Z = PV*(1/den) + Qtok (DVE), LN deferred.

LN rstd = Exp(-0.5*Ln(var+eps)) so the Act engine keeps one table
(natural_log_exp_and_others) loaded for the entire kernel - no act-table
swaps between softmax exps and LN.

g0/b0 folded into W1/W2/fcb on host. Zero biases / identity final affine
(always true for this problem's setup_inputs) are specialized away at
build time from the actual runtime inputs; a general fallback variant is
built otherwise.
"""

import sys

sys.path.insert(0, "/opt/trn_rl_repo")

import numpy as np
import ml_dtypes

import concourse.bass as bass
import concourse.tile as tile
from concourse import bacc, mybir
from concourse.bass_utils import run_bass_kernel_spmd

N = 1024  # tokens per sequence
D = 1024  # model dim
H = 8  # heads
DH = 128  # head dim
P = 128  # partitions
NT = N // P  # 8 token tiles
FT = D // P  # 8 feature tiles
EPS = 1e-5
F8 = mybir.dt.float8e4
F16 = mybir.dt.float16
F32 = mybir.dt.float32
NF8 = ml_dtypes.float8_e4m3
SCALE = 1.0 / 32.0  # 1/sqrt(D)
DR = mybir.MatmulPerfMode.DoubleRow

_CACHED = {}


def _pin_act_tables(arch):
    """Make natural_log_exp_and_others the only table containing Exp/Ln so
    the act-table-load inserter emits one load for the whole kernel instead
    of thrashing between the exp and ln tables. The emitted load still
    references the real table (which genuinely holds both functions)."""
    from concourse.hw_specs import get_activation_tables

    try:
        tables = get_activation_tables(arch)
    except Exception:
        return
    keep = "natural_log_exp_and_others"
    if keep not in tables:
        return
    for name, funcs in tables.items():
        if name != keep:
            funcs.discard(mybir.ActivationFunctionType.Exp)
            funcs.discard(mybir.ActivationFunctionType.Ln)


def _bcast_ap(vec_ap, cols):
    """[cols]-element DRAM vector -> [128, cols] partition-broadcast AP."""
    return bass.AP(
        tensor=vec_ap.tensor, offset=vec_ap.offset, ap=[[0, P], [1, cols]]
    )


def _pair(ap2d, stride):
    """[128, n] AP -> [128, 2, n] inserting a dim-1 of the given stride."""
    return bass.AP(
        tensor=ap2d.tensor, offset=ap2d.offset,
        ap=[ap2d.ap[0], [stride, 2], ap2d.ap[-1]],
    )


def _build(general):
    nc = bacc.Bacc(None, target_bir_lowering=False, debug=False)
    if not _CACHED.get("no_pin"):
        _pin_act_tables(nc.m.arch)

    dram = {}
    # transposed activations (host-side transpose): [D, N]
    dram["at"] = nc.dram_tensor("at", (D, N), F16, kind="ExternalInput")
    dram["at8"] = nc.dram_tensor("at8", (D, N), F8, kind="ExternalInput")
    dram["ct8"] = nc.dram_tensor("ct8", (D, N), F8, kind="ExternalInput")
    dram["wq"] = nc.dram_tensor("wq", (D, D), F16, kind="ExternalInput")
    for nm in ("wk1", "wk2", "wv1", "wv2"):
        dram[nm] = nc.dram_tensor(nm, (D, D), F8, kind="ExternalInput")
    for nm in ("w1", "w2"):
        dram[nm] = nc.dram_tensor(nm, (D, D), F16, kind="ExternalInput")
    if general:
        for nm in ("bq", "bk1", "bk2", "bv1", "bv2", "fcb", "g1v", "b1v"):
            dram[nm] = nc.dram_tensor(nm, (D,), F32, kind="ExternalInput")
    o_dram = nc.dram_tensor("o", (N, D), F32, kind="ExternalOutput")

    with tile.TileContext(nc) as tc:
        import contextlib

        ctx = contextlib.ExitStack()
        with ctx:
            const = ctx.enter_context(tc.tile_pool(name="const", bufs=1))
            small = ctx.enter_context(tc.tile_pool(name="small", bufs=8))
            eps_tile = const.tile([P, 1], F32, tag="eps")
            nc.vector.memset(eps_tile[:], EPS)

            bias_sb = {}
            if general:
                for nm in ("bq", "bk1", "bk2"):
                    t = const.tile([P, FT], F32, tag=f"{nm}_sb")
                    nc.sync.dma_start(
                        t[:], dram[nm][:].rearrange("(t p) -> p t", p=P)
                    )
                    bias_sb[nm] = t
                for nm in ("bv1", "bv2", "fcb", "g1v", "b1v"):
                    t = const.tile([P, D], F32, tag=f"{nm}_bc")
                    nc.sync.dma_start(t[:], _bcast_ap(dram[nm][:], D))
                    bias_sb[nm] = t

            # ---------------- persistent SBUF ----------------
            persist = ctx.enter_context(tc.tile_pool(name="persist", bufs=1))
            lt = {
                "own": persist.tile([P, FT, N], F16, tag="lto", name="lt_own"),
                "oth": persist.tile([P, FT, N], F16, tag="ltx", name="lt_oth"),
            }
            attn_data = tc.alloc_tile_pool(name="attn_data", bufs=1)
            ctx.callback(attn_data.release)
            # qzz row FT is all zeros: the moving zero group for DR QK
            qzz = attn_data.tile([P, FT + 1, N], F8, tag="qzz", name="qzz")
            k1t = attn_data.tile([P, H, N], F8, tag="k1t", name="k1t")
            k2t = attn_data.tile([P, H, N], F8, tag="k2t", name="k2t")
            v1 = attn_data.tile([P, NT, H, DH + 1], F8, tag="v1", name="v1")
            v2 = attn_data.tile([P, NT, H, DH + 1], F8, tag="v2", name="v2")
            qtok = attn_data.tile([P, NT, D], F16, tag="qtok", name="qtok")
            zc = {
                "own": attn_data.tile([P, NT, D], F16, tag="zco", name="zc_own"),
                "oth": attn_data.tile([P, NT, D], F16, tag="zcx", name="zc_oth"),
            }
            nc.gpsimd.memset(qzz[:, FT, :], 0.0)
            nc.gpsimd.memset(v1[:, :, :, DH], 1.0)
            nc.gpsimd.memset(v2[:, :, :, DH], 1.0)

            # FC weights: allocated early (LIFO), DMA'd after block1 panels
            fc_w = tc.alloc_tile_pool(name="fc_w", bufs=1)
            ctx.callback(fc_w.release)
            wps = {}
            for oc in range(2):
                for nm in ("w1", "w2"):
                    wps[(nm, oc)] = fc_w.tile(
                        [P, FT, 512], F16, tag=f"wf_{nm}{oc}", name=f"wf_{nm}{oc}"
                    )

            # phase-A inputs on the right-side pool stack: at lives through
            # block2 (Q ic1 projs), at8 only through block1, ct8 swaps into
            # at8's space afterwards.
            pool_at = tc.alloc_tile_pool(name="pool_at", bufs=1, side="right")
            at = pool_at.tile([P, FT, N], F16, tag="at", name="at_all")
            pool_a8 = tc.alloc_tile_pool(name="pool_a8", bufs=1, side="right")
            at8 = pool_a8.tile([P, FT, N], F8, tag="at8", name="at8_all")
            nc.sync.dma_start(at8[:], dram["at8"][:].rearrange("(t p) n -> p t n", p=P))
            nc.sync.dma_start(at[:], dram["at"][:].rearrange("(t p) n -> p t n", p=P))

            # rings
            wq_pan = ctx.enter_context(tc.tile_pool(name="wq_pan", bufs=2))
            wk_pan = ctx.enter_context(tc.tile_pool(name="wk_pan", bufs=2))
            wv_pan = ctx.enter_context(tc.tile_pool(name="wv_pan", bufs=2))
            q16_pan = ctx.enter_context(tc.tile_pool(name="q16_pan", bufs=2))
            p_pool = ctx.enter_context(tc.tile_pool(name="p_pool", bufs=6))
            ltok_pool = ctx.enter_context(tc.tile_pool(name="ltok_pool", bufs=2))
            u_pool = ctx.enter_context(tc.tile_pool(name="u_pool", bufs=2))
            out_pool = ctx.enter_context(tc.tile_pool(name="out_pool", bufs=1))
            psum_s = tc.alloc_tile_pool(name="psum_s", bufs=2, space="PSUM")
            psum_pv = tc.alloc_tile_pool(name="psum_pv", bufs=2, space="PSUM")
            psum_pr = tc.alloc_tile_pool(name="psum_pr", bufs=2, space="PSUM")

            def ln_rstd(src_ap):
                """LN stats of src_ap [128, D] -> (mean ap, rstd ap).
                rstd = Exp(-0.5 * Ln(var + eps)): keeps the Act engine on the
                natural_log_exp table (no swaps against softmax exps)."""
                stats = small.tile([P, 2, 6], F32, tag="ln_stats")
                for sg in range(2):
                    nc.vector.bn_stats(
                        out=stats[:, sg, :], in_=src_ap[:, sg * 512: (sg + 1) * 512]
                    )
                mv = small.tile([P, 2], F32, tag="ln_mv")
                nc.vector.bn_aggr(out=mv[:], in_=stats[:])
                lnv = small.tile([P, 1], F32, tag="ln_lnv")
                nc.scalar.activation(
                    out=lnv[:], in_=mv[:, 1:2],
                    func=mybir.ActivationFunctionType.Ln,
                    bias=eps_tile[:], scale=1.0,
                )
                rstd = small.tile([P, 1], F32, tag="ln_rstd")
                nc.scalar.activation(
                    out=rstd[:], in_=lnv[:],
                    func=mybir.ActivationFunctionType.Exp,
                    scale=-0.5,
                )
                return mv[:, 0:1], rstd[:]

            def ln_aggr(stats):
                """bn_aggr + rstd from a precomputed [P,2,6] stats tile."""
                mv = small.tile([P, 2], F32, tag="ln_mv")
                nc.vector.bn_aggr(out=mv[:], in_=stats[:])
                lnv = small.tile([P, 1], F32, tag="ln_lnv")
                nc.scalar.activation(
                    out=lnv[:], in_=mv[:, 1:2],
                    func=mybir.ActivationFunctionType.Ln,
                    bias=eps_tile[:], scale=1.0,
                )
                rstd = small.tile([P, 1], F32, tag="ln_rstd")
                nc.scalar.activation(
                    out=rstd[:], in_=lnv[:],
                    func=mybir.ActivationFunctionType.Exp,
                    scale=-0.5,
                )
                return mv[:, 0:1], rstd[:]

            def kproj(h, w_dram, src8, dst, bias_key):
                """dst[:, h, :] = (W[:, h-feats]^T src8)  (fp8 DoubleRow)."""
                wp = wk_pan.tile([P, FT, P], F8, tag="wp8")
                nc.sync.dma_start(
                    wp[:],
                    w_dram[:, h * P: (h + 1) * P].rearrange("(dt p) f -> p dt f", p=P),
                )
                for ic in range(2):
                    ps = psum_pr.tile([P, 512], F32, tag="psA")
                    for d in range(FT // 2):
                        nc.tensor.matmul(
                            ps[:],
                            wp[:, 2 * d: 2 * d + 2, :],
                            src8[:, 2 * d: 2 * d + 2, ic * 512: (ic + 1) * 512],
                            start=(d == 0), stop=(d == FT // 2 - 1),
                            perf_mode=DR,
                        )
                    out = dst[:, h, ic * 512: (ic + 1) * 512]
                    if general:
                        nc.vector.tensor_scalar(
                            out=out, in0=ps[:],
                            scalar1=bias_sb[bias_key][:, h: h + 1],
                            scalar2=None, op0=mybir.AluOpType.add,
                        )
                    else:
                        nc.vector.tensor_copy(out, ps[:])

            def qproj(h, ic):
                """qzz[:, h, ic-half], qtok = A @ Wq[:, h-feats] (f16) for
                one query half (panel re-fetched per half)."""
                wp = wq_pan.tile([P, FT, P], F16, tag="wp16")
                nc.sync.dma_start(
                    wp[:],
                    dram["wq"][:, h * P: (h + 1) * P].rearrange(
                        "(dt p) f -> p dt f", p=P
                    ),
                )
                if True:
                    ps = psum_pr.tile([P, 512], F32, tag="psA")
                    for d in range(FT):
                        nc.tensor.matmul(
                            ps[:],
                            wp[:, d, :],
                            at[:, d, ic * 512: (ic + 1) * 512],
                            start=(d == 0), stop=(d == FT - 1),
                        )
                    q16 = q16_pan.tile([P, 512], F16, tag="q16")
                    if general:
                        nc.vector.tensor_scalar(
                            out=q16[:], in0=ps[:],
                            scalar1=bias_sb["bq"][:, h: h + 1],
                            scalar2=None, op0=mybir.AluOpType.add,
                        )
                    else:
                        nc.vector.tensor_copy(q16[:], ps[:])
                    nc.gpsimd.tensor_copy(qzz[:, h, ic * 512: (ic + 1) * 512], q16[:])
                    nc.sync.dma_start_transpose(
                        qtok[:, ic * 4: (ic + 1) * 4, h * P: (h + 1) * P], q16[:]
                    )

            def vproj(h, w_dram, src8, dst, bias_key):
                """dst[:, :, h, 0:DH] = src8^T @ W[:, h-feats]  (fp8 DR)."""
                wp = wv_pan.tile([P, FT, P], F8, tag="wv8")
                nc.sync.dma_start(
                    wp[:],
                    w_dram[:, h * P: (h + 1) * P].rearrange("(dt p) f -> p dt f", p=P),
                )
                for jh in range(2):
                    ps0 = psum_pr.tile([P, 512], F32, tag="psA")
                    ps = ps0[:].rearrange("p (j f) -> p j f", f=P)
                    for j in range(4):
                        for d in range(FT // 2):
                            nc.tensor.matmul(
                                ps[:, j, :],
                                src8[:, 2 * d: 2 * d + 2,
                                     (4 * jh + j) * P: (4 * jh + j + 1) * P],
                                wp[:, 2 * d: 2 * d + 2, :],
                                start=(d == 0), stop=(d == FT // 2 - 1),
                                perf_mode=DR,
                            )
                    out = dst[:, 4 * jh: 4 * jh + 4, h, 0:DH]
                    if general:
                        bc = bias_sb[bias_key][:, h * DH: (h + 1) * DH]
                        bc4 = bass.AP(tensor=bc.tensor, offset=bc.offset,
                                      ap=[bc.ap[0], [0, 4], bc.ap[-1]])
                        nc.vector.tensor_add(out, ps, bc4)
                    else:
                        nc.vector.tensor_copy(out, ps)

            def attn_qk(key, kt, ic, h):
                """QK + exp for one head/half; returns the P tiles."""
                base = qzz[:, h, ic * 512: (ic + 1) * 512]
                qmov = _pair(base, (FT - h) * N)
                p_t = []
                for jp in range(4):
                    sps = psum_s.tile([P, 2, 512], F32, tag="sps")
                    for half in range(2):
                        jt = 2 * jp + half
                        nc.tensor.matmul(
                            sps[:, half, :],
                            _pair(kt[:, h, jt * P: (jt + 1) * P], 0),
                            qmov,
                            start=True, stop=True,
                            perf_mode=DR,
                        )
                    pt = p_pool.tile([P, 2, 512], F8, tag="p_t", name="p_t")
                    nc.scalar.activation(
                        out=pt[:], in_=sps[:],
                        func=mybir.ActivationFunctionType.Exp,
                        scale=SCALE,
                    )
                    p_t.append(pt)
                return p_t

            def attn_pv(key, v_t, ic, h, p_t):
                """PV + denominator + residual z-writes for one head/half."""
                for ip in range(2):
                    pvp = psum_pv.tile([P, 2, DH + 1], F32, tag="pvp")
                    for half in range(2):
                        il = 2 * ip + half
                        for jp in range(4):
                            nc.tensor.matmul(
                                pvp[:, half, :],
                                p_t[jp][:, :, il * P: (il + 1) * P],
                                v_t[:, 2 * jp: 2 * jp + 2, h, :],
                                start=(jp == 0), stop=(jp == 3),
                                perf_mode=DR,
                            )
                    rcp = small.tile([P, 2], F32, tag="rcp")
                    nc.vector.reciprocal(rcp[:], pvp[:, :, DH])
                    for half in range(2):
                        il = 2 * ip + half
                        it = ic * 4 + il
                        nc.vector.scalar_tensor_tensor(
                            out=zc[key][:, it, h * DH: (h + 1) * DH],
                            in0=pvp[:, half, 0:DH],
                            scalar=rcp[:, half: half + 1],
                            in1=qtok[:, it, h * DH: (h + 1) * DH],
                            op0=mybir.AluOpType.mult,
                            op1=mybir.AluOpType.add,
                        )

            def emit_ln(key, it):
                """LN zc[key][:, it, :] -> lt[key] feature-major (transpose)."""
                mean, rstd = ln_rstd(zc[key][:, it, :])
                ltok = ltok_pool.tile([P, D], F16, tag="ltok")
                nc.gpsimd.tensor_scalar(
                    out=ltok[:], in0=zc[key][:, it, :],
                    scalar1=mean, scalar2=rstd,
                    op0=mybir.AluOpType.subtract, op1=mybir.AluOpType.mult,
                )
                nc.sync.dma_start_transpose(
                    lt[key][:, :, it * P: (it + 1) * P], ltok[:]
                )

            # ---------------- block 1: per-h K1,Q(ic0),V1,QK; PV deferred ---
            pend = None  # (key, v_t, ic, h, p_t)
            for h in range(H):
                kproj(h, dram["wk1"], at8, k1t, "bk1")
                qproj(h, 0)
                vproj(h, dram["wv1"], at8, v1, "bv1")
                p_t = attn_qk("own", k1t, 0, h)
                if pend is not None:
                    attn_pv(*pend)
                pend = ("own", v1, 0, h, p_t)

            # at8 -> ct8 space swap; FC weights stream on SWDGE
            pool_a8.release()
            pool_c = tc.alloc_tile_pool(name="pool_c", bufs=1, side="right")
            ct8 = pool_c.tile([P, FT, N], F8, tag="ct8", name="ct8_all")
            nc.sync.dma_start(ct8[:], dram["ct8"][:].rearrange("(t p) n -> p t n", p=P))
            for oc in range(2):
                for nm in ("w1", "w2"):
                    nc.gpsimd.dma_start(
                        wps[(nm, oc)][:],
                        dram[nm][:, oc * 512: (oc + 1) * 512].rearrange(
                            "(dt p) f -> p dt f", p=P
                        ),
                    )

            # ---------------- block 2: per-h K2,V2,Q(ic1),QK; PV deferred ---
            for h in range(H):
                kproj(h, dram["wk2"], ct8, k2t, "bk2")
                vproj(h, dram["wv2"], ct8, v2, "bv2")
                qproj(h, 1)
                p_t = attn_qk("oth", k2t, 0, h)
                attn_pv(*pend)
                pend = ("oth", v2, 0, h, p_t)
            attn_pv(*pend)
            pend = None
            pool_c.release()
            pool_at.release()

            # proj psum banks -> FC psum
            psum_pr.release()
            psum_fc = tc.alloc_tile_pool(name="psum_fc", bufs=2, space="PSUM")

            # LN token tiles 0-3 (both attns) while block 3 runs
            for it in range(4):
                emit_ln("own", it)
                emit_ln("oth", it)

            # --------- FC machinery: 16 matmuls per (it, oc) group ----------
            fc_state = {}

            def fc_half(idx):
                """Half-group idx: 8 of the 16 matmuls of group g = idx//2."""
                g = idx // 2
                it, oc = g // 2, g % 2
                key, wnm = (("own", "w1") if idx % 2 == 0 else ("oth", "w2"))
                if idx % 2 == 0:
                    fc_state[g] = psum_fc.tile([P, 512], F32, tag="fps", name="fps")
                    if oc == 0:
                        fc_state[("u", it)] = u_pool.tile(
                            [P, D], F16, tag="ut", name="ut"
                        )
                fps = fc_state[g]
                for kt_i in range(FT):
                    nc.tensor.matmul(
                        fps[:],
                        lt[key][:, kt_i, it * P: (it + 1) * P],
                        wps[(wnm, oc)][:, kt_i, :],
                        start=(idx % 2 == 0 and kt_i == 0),
                        stop=(idx % 2 == 1 and kt_i == FT - 1),
                    )
                if idx % 2 == 1:
                    ut = fc_state[("u", it)]
                    if general:
                        nc.vector.scalar_tensor_tensor(
                            out=ut[:, oc * 512: (oc + 1) * 512],
                            in0=fps[:], scalar=1.0,
                            in1=bias_sb["fcb"][:, oc * 512: (oc + 1) * 512],
                            op0=mybir.AluOpType.mult, op1=mybir.AluOpType.add,
                        )
                    else:
                        nc.vector.tensor_scalar(
                            out=ut[:, oc * 512: (oc + 1) * 512],
                            in0=fps[:], scalar1=0.0, scalar2=None,
                            op0=mybir.AluOpType.max,
                        )
                        # u-LN stats for this half right away (hides the
                        # bn_stats latency behind the other oc's matmuls)
                        if oc == 0:
                            fc_state[("st", it)] = small.tile(
                                [P, 2, 6], F32, tag="u_stats", name="ust"
                            )
                        nc.vector.bn_stats(
                            out=fc_state[("st", it)][:, oc, :],
                            in_=ut[:, oc * 512: (oc + 1) * 512],
                        )
                    if oc == 1:
                        finish_token(it)

            def finish_token(it):
                """relu'd ut complete: final LN + store."""
                ut = fc_state.pop(("u", it))
                if general:
                    nc.gpsimd.tensor_scalar(
                        out=ut[:], in0=ut[:], scalar1=0.0, scalar2=None,
                        op0=mybir.AluOpType.max,
                    )
                    mean, rstd = ln_rstd(ut[:])
                else:
                    mean, rstd = ln_aggr(fc_state.pop(("st", it)))
                ot = out_pool.tile([P, D], F32, tag="ot")
                nc.gpsimd.tensor_scalar(
                    out=ot[:], in0=ut[:], scalar1=mean, scalar2=rstd,
                    op0=mybir.AluOpType.subtract, op1=mybir.AluOpType.mult,
                )
                if general:
                    nc.vector.tensor_mul(ot[:], ot[:], bias_sb["g1v"][:])
                    nc.vector.tensor_add(ot[:], ot[:], bias_sb["b1v"][:])
                nc.sync.dma_start(o_dram[it * P: (it + 1) * P, :], ot[:])

            # ---------------- block 3: own attn ic1 + FC halves 0-5 ---------
            for h in range(H):
                p_t = attn_qk("own", k1t, 1, h)
                if pend is not None:
                    attn_pv(*pend)
                if h >= 2:
                    fc_half(h - 2)
                pend = ("own", v1, 1, h, p_t)

            # ------- block 4: oth attn ic1 + FC 6-15 + LN(own,4-7) ----------
            for h in range(H):
                p_t = attn_qk("oth", k2t, 1, h)
                attn_pv(*pend)
                if h < 6:
                    fc_half(6 + h)
                else:
                    fc_half(2 * h)       # h6 -> 12, h7 -> 14
                    fc_half(2 * h + 1)   # h6 -> 13, h7 -> 15
                if h < 4:
                    emit_ln("own", 4 + h)  # token tiles 4..7 (zc own done)
                pend = ("oth", v2, 1, h, p_t)
            attn_pv(*pend)

            # ---------------- tail: its 4-7, own-halves lead ----------------
            emit_ln("oth", 4)
            emit_ln("oth", 5)
            fc_half(16); fc_half(18); fc_half(17); fc_half(19)
            emit_ln("oth", 6)
            fc_half(20); fc_half(22); fc_half(21); fc_half(23)
            emit_ln("oth", 7)
            fc_half(24); fc_half(26); fc_half(25); fc_half(27)
            fc_half(28); fc_half(30); fc_half(29); fc_half(31)

            psum_fc.release()
            psum_pv.release()
            psum_s.release()

    nc.compile()
    return nc


def build_in_maps(X, Y, Wqx, bqx, Wkx, bkx, Wvx, bvx, Wqy, bqy, Wky, bky,
                  Wvy, bvy, WX, bX, WY, bY, g0, b0, g1, b1):
    f = lambda t: np.asarray(t, dtype=np.float32)
    h16 = lambda t: np.ascontiguousarray(f(t).astype(np.float16))
    h8 = lambda t: np.ascontiguousarray(f(t).astype(np.float16).astype(NF8))
    X, Y = f(X), f(Y)
    g0d, b0d = f(g0).astype(np.float64), f(b0).astype(np.float64)

    general = not (
        all(np.allclose(f(b), 0.0) for b in (bqx, bkx, bvx, bqy, bky, bvy,
                                             bX, bY, b0, b1))
        and np.allclose(f(g1), 1.0)
    )

    sides = {}
    for side, W, bo in (("x", f(WX), f(bX)), ("y", f(WY), f(bY))):
        Wtop = W[:D].astype(np.float64)
        Wbot = W[D:].astype(np.float64)
        fcb = (b0d @ Wtop + b0d @ Wbot + bo.astype(np.float64)).astype(np.float32)
        w_top_folded = (g0d[:, None] * Wtop).astype(np.float32)
        w_bot_folded = (g0d[:, None] * Wbot).astype(np.float32)
        if side == "x":
            w_own, w_oth = w_top_folded, w_bot_folded  # [O_xx, O_xy]
        else:
            w_own, w_oth = w_bot_folded, w_top_folded  # [O_yx, O_yy]
        sides[side] = dict(w1=h16(w_own), w2=h16(w_oth), fcb=fcb)

    wx = dict(wq=h16(Wqx), bq=f(bqx), wk=h8(Wkx), bk=f(bkx), wv=h8(Wvx),
              bv=f(bvx))
    wy = dict(wq=h16(Wqy), bq=f(bqy), wk=h8(Wky), bk=f(bky), wv=h8(Wvy),
              bv=f(bvy))

    seq_t = {}
    for nm, S in (("x", X), ("y", Y)):
        for b in range(S.shape[0]):
            t16 = np.ascontiguousarray(S[b].T.astype(np.float16))
            seq_t[(nm, b)] = (t16, np.ascontiguousarray(t16.astype(NF8)))

    in_maps = []
    for core in range(8):
        b = core // 2
        side = "x" if core % 2 == 0 else "y"
        own, oth = (wx, wy) if side == "x" else (wy, wx)
        a_t = seq_t[(side, b)]
        c_t = seq_t[("y" if side == "x" else "x", b)]
        m = {
            "at": a_t[0], "at8": a_t[1], "ct8": c_t[1],
            "wq": own["wq"],
            "wk1": own["wk"], "wv1": own["wv"],
            "wk2": oth["wk"], "wv2": oth["wv"],
            "w1": sides[side]["w1"], "w2": sides[side]["w2"],
        }
        if general:
            m.update({
                "bq": own["bq"], "bk1": own["bk"], "bv1": own["bv"],
                "bk2": oth["bk"], "bv2": oth["bv"],
                "fcb": sides[side]["fcb"], "g1v": f(g1), "b1v": f(b1),
            })
        in_maps.append(m)
    return in_maps, general


def kernel(**inputs):
    import time as _time

    in_maps, general = build_in_maps(**inputs)
    key = ("nc", general)
    if key not in _CACHED:
        _CACHED[key] = _build(general)
    nc = _CACHED[key]
    _CACHED["nc"] = nc  # test.py compatibility

    res = None
    for attempt in range(4):
        try:
            res = run_bass_kernel_spmd(nc, in_maps, list(range(8)))
            break
        except Exception:
            if attempt == 3:
                raise
            _time.sleep(2.0)
    _CACHED["last_result"] = res

    B = np.asarray(inputs["X"]).shape[0]
    O_x = np.stack([res.results[2 * b]["o"] for b in range(B)])
    O_y = np.stack([res.results[2 * b + 1]["o"] for b in range(B)])
    return O_x, O_y


# revision 39
# speedup vs baseline: 1.7707x; 1.0532x over previous
"""Trainium2 Bass kernel for nn_CSAB2 (cross-set attention block, 8 cores).

Sharding: zero-collective. 8 cores = 4 batches x 2 output sides (x / y).
Each core computes one full output O_x[b] or O_y[b] (1024, 1024).

Mixed fp8/fp16 design, software-pipelined around the Act-engine exp spine
(16.8M softmax exps/core are the irreducible Act-bound phase):

  block1 (own ic0):  per head h: K1(h) fp8-DR proj, Q(h) f16 proj,
                     V1(h) fp8-DR proj, then attention(own, ic0, h)
  block2 (oth ic0):  per head: K2(h), V2(h) projections + attention
  LN+transpose of token tiles 0-3 (both attns)
  block3 (own ic1):  attention + interleaved FC matmul half-groups
  block4 (oth ic1):  attention + FC + LN(own, tiles 4-7)
  tail:              LN(oth, 4-7), FC(4-7), relu, final LN, store

Attention per (ic, h): St = K_h^T.DR.[Q8|0] (stride-0 stationary pair,
moving zero row in qzz), P = exp(St/32) -> fp8, PV via fp8-DR over
paired j-tiles with an appended ones column (denominator for free),
Z = PV*(1/den) + Qtok (DVE), LN deferred.

LN rstd = Exp(-0.5*Ln(var+eps)) so the Act engine keeps one table
(natural_log_exp_and_others) loaded for the entire kernel - no act-table
swaps between softmax exps and LN.

g0/b0 folded into W1/W2/fcb on host. Zero biases / identity final affine
(always true for this problem's setup_inputs) are specialized away at
build time from the actual runtime inputs; a general fallback variant is
built otherwise.
"""

import sys

sys.path.insert(0, "/opt/trn_rl_repo")

import numpy as np
import ml_dtypes

import concourse.bass as bass
import concourse.tile as tile
from concourse import bacc, mybir
from concourse.bass_utils import run_bass_kernel_spmd

N = 1024  # tokens per sequence
D = 1024  # model dim
H = 8  # heads
DH = 128  # head dim
P = 128  # partitions
NT = N // P  # 8 token tiles
FT = D // P  # 8 feature tiles
EPS = 1e-5
F8 = mybir.dt.float8e4
F16 = mybir.dt.float16
F32 = mybir.dt.float32
NF8 = ml_dtypes.float8_e4m3
SCALE = 1.0 / 32.0  # 1/sqrt(D)
DR = mybir.MatmulPerfMode.DoubleRow

_CACHED = {}


def _pin_act_tables(arch):
    """Make natural_log_exp_and_others the only table containing Exp/Ln so
    the act-table-load inserter emits one load for the whole kernel instead
    of thrashing between the exp and ln tables. The emitted load still
    references the real table (which genuinely holds both functions)."""
    from concourse.hw_specs import get_activation_tables

    try:
        tables = get_activation_tables(arch)
    except Exception:
        return
    keep = "natural_log_exp_and_others"
    if keep not in tables:
        return
    for name, funcs in tables.items():
        if name != keep:
            funcs.discard(mybir.ActivationFunctionType.Exp)
            funcs.discard(mybir.ActivationFunctionType.Ln)


def _bcast_ap(vec_ap, cols):
    """[cols]-element DRAM vector -> [128, cols] partition-broadcast AP."""
    return bass.AP(
        tensor=vec_ap.tensor, offset=vec_ap.offset, ap=[[0, P], [1, cols]]
    )


def _pair(ap2d, stride):
    """[128, n] AP -> [128, 2, n] inserting a dim-1 of the given stride."""
    return bass.AP(
        tensor=ap2d.tensor, offset=ap2d.offset,
        ap=[ap2d.ap[0], [stride, 2], ap2d.ap[-1]],
    )


def _build(general):
    nc = bacc.Bacc(None, target_bir_lowering=False, debug=False)
    if not _CACHED.get("no_pin"):
        _pin_act_tables(nc.m.arch)

    dram = {}
    # transposed activations (host-side transpose): [D, N]
    dram["at"] = nc.dram_tensor("at", (D, N), F16, kind="ExternalInput")
    dram["at8"] = nc.dram_tensor("at8", (D, N), F8, kind="ExternalInput")
    dram["ct8"] = nc.dram_tensor("ct8", (D, N), F8, kind="ExternalInput")
    dram["wq"] = nc.dram_tensor("wq", (D, D), F16, kind="ExternalInput")
    for nm in ("wk1", "wk2", "wv1", "wv2"):
        dram[nm] = nc.dram_tensor(nm, (D, D), F8, kind="ExternalInput")
    for nm in ("w1", "w2"):
        dram[nm] = nc.dram_tensor(nm, (D, D), F16, kind="ExternalInput")
    if general:
        for nm in ("bq", "bk1", "bk2", "bv1", "bv2", "fcb", "g1v", "b1v"):
            dram[nm] = nc.dram_tensor(nm, (D,), F32, kind="ExternalInput")
    o_dram = nc.dram_tensor("o", (N, D), F32, kind="ExternalOutput")

    with tile.TileContext(nc) as tc:
        import contextlib

        ctx = contextlib.ExitStack()
        with ctx:
            const = ctx.enter_context(tc.tile_pool(name="const", bufs=1))
            small = ctx.enter_context(tc.tile_pool(name="small", bufs=8))
            eps_tile = const.tile([P, 1], F32, tag="eps")
            nc.vector.memset(eps_tile[:], EPS)

            bias_sb = {}
            if general:
                for nm in ("bq", "bk1", "bk2"):
                    t = const.tile([P, FT], F32, tag=f"{nm}_sb")
                    nc.sync.dma_start(
                        t[:], dram[nm][:].rearrange("(t p) -> p t", p=P)
                    )
                    bias_sb[nm] = t
                for nm in ("bv1", "bv2", "fcb", "g1v", "b1v"):
                    t = const.tile([P, D], F32, tag=f"{nm}_bc")
                    nc.sync.dma_start(t[:], _bcast_ap(dram[nm][:], D))
                    bias_sb[nm] = t

            # ---------------- persistent SBUF ----------------
            persist = ctx.enter_context(tc.tile_pool(name="persist", bufs=1))
            lt = {
                "own": persist.tile([P, FT, N], F16, tag="lto", name="lt_own"),
                "oth": persist.tile([P, FT, N], F16, tag="ltx", name="lt_oth"),
            }
            attn_data = tc.alloc_tile_pool(name="attn_data", bufs=1)
            ctx.callback(attn_data.release)
            # qzz row FT is all zeros: the moving zero group for DR QK
            qzz = attn_data.tile([P, FT + 1, N], F8, tag="qzz", name="qzz")
            k1t = attn_data.tile([P, H, N], F8, tag="k1t", name="k1t")
            k2t = attn_data.tile([P, H, N], F8, tag="k2t", name="k2t")
            v1 = attn_data.tile([P, NT, H, DH + 1], F8, tag="v1", name="v1")
            v2 = attn_data.tile([P, NT, H, DH + 1], F8, tag="v2", name="v2")
            qtok = attn_data.tile([P, NT, D], F16, tag="qtok", name="qtok")
            zc = {
                "own": attn_data.tile([P, NT, D], F16, tag="zco", name="zc_own"),
                "oth": attn_data.tile([P, NT, D], F16, tag="zcx", name="zc_oth"),
            }
            nc.gpsimd.memset(qzz[:, FT, :], 0.0)
            nc.gpsimd.memset(v1[:, :, :, DH], 1.0)
            nc.gpsimd.memset(v2[:, :, :, DH], 1.0)

            # FC weights: allocated early (LIFO), DMA'd after block1 panels
            fc_w = tc.alloc_tile_pool(name="fc_w", bufs=1)
            ctx.callback(fc_w.release)
            wps = {}
            for oc in range(2):
                for nm in ("w1", "w2"):
                    wps[(nm, oc)] = fc_w.tile(
                        [P, FT, 512], F16, tag=f"wf_{nm}{oc}", name=f"wf_{nm}{oc}"
                    )

            # phase-A inputs on the right-side pool stack: at lives through
            # block2 (Q ic1 projs), at8 only through block1, ct8 swaps into
            # at8's space afterwards.
            pool_at = tc.alloc_tile_pool(name="pool_at", bufs=1, side="right")
            at = pool_at.tile([P, FT, N], F16, tag="at", name="at_all")
            pool_a8 = tc.alloc_tile_pool(name="pool_a8", bufs=1, side="right")
            at8 = pool_a8.tile([P, FT, N], F8, tag="at8", name="at8_all")
            nc.sync.dma_start(at8[:], dram["at8"][:].rearrange("(t p) n -> p t n", p=P))

            # rings
            wq_pan = ctx.enter_context(tc.tile_pool(name="wq_pan", bufs=2))
            wk_pan = ctx.enter_context(tc.tile_pool(name="wk_pan", bufs=2))
            wv_pan = ctx.enter_context(tc.tile_pool(name="wv_pan", bufs=2))
            q16_pan = ctx.enter_context(tc.tile_pool(name="q16_pan", bufs=2))
            p_pool = ctx.enter_context(tc.tile_pool(name="p_pool", bufs=6))
            ltok_pool = ctx.enter_context(tc.tile_pool(name="ltok_pool", bufs=2))
            u_pool = ctx.enter_context(tc.tile_pool(name="u_pool", bufs=2))
            out_pool = ctx.enter_context(tc.tile_pool(name="out_pool", bufs=1))
            psum_s = tc.alloc_tile_pool(name="psum_s", bufs=2, space="PSUM")
            psum_pv = tc.alloc_tile_pool(name="psum_pv", bufs=2, space="PSUM")
            psum_pr = tc.alloc_tile_pool(name="psum_pr", bufs=2, space="PSUM")

            def ln_rstd(src_ap):
                """LN stats of src_ap [128, D] -> (mean ap, rstd ap).
                rstd = Exp(-0.5 * Ln(var + eps)): keeps the Act engine on the
                natural_log_exp table (no swaps against softmax exps)."""
                stats = small.tile([P, 2, 6], F32, tag="ln_stats")
                for sg in range(2):
                    nc.vector.bn_stats(
                        out=stats[:, sg, :], in_=src_ap[:, sg * 512: (sg + 1) * 512]
                    )
                mv = small.tile([P, 2], F32, tag="ln_mv")
                nc.vector.bn_aggr(out=mv[:], in_=stats[:])
                lnv = small.tile([P, 1], F32, tag="ln_lnv")
                nc.scalar.activation(
                    out=lnv[:], in_=mv[:, 1:2],
                    func=mybir.ActivationFunctionType.Ln,
                    bias=eps_tile[:], scale=1.0,
                )
                rstd = small.tile([P, 1], F32, tag="ln_rstd")
                nc.scalar.activation(
                    out=rstd[:], in_=lnv[:],
                    func=mybir.ActivationFunctionType.Exp,
                    scale=-0.5,
                )
                return mv[:, 0:1], rstd[:]

            def ln_aggr(stats):
                """bn_aggr + rstd from a precomputed [P,2,6] stats tile."""
                mv = small.tile([P, 2], F32, tag="ln_mv")
                nc.vector.bn_aggr(out=mv[:], in_=stats[:])
                lnv = small.tile([P, 1], F32, tag="ln_lnv")
                nc.scalar.activation(
                    out=lnv[:], in_=mv[:, 1:2],
                    func=mybir.ActivationFunctionType.Ln,
                    bias=eps_tile[:], scale=1.0,
                )
                rstd = small.tile([P, 1], F32, tag="ln_rstd")
                nc.scalar.activation(
                    out=rstd[:], in_=lnv[:],
                    func=mybir.ActivationFunctionType.Exp,
                    scale=-0.5,
                )
                return mv[:, 0:1], rstd[:]

            def kproj(h, w_dram, src8, dst, bias_key):
                """dst[:, h, :] = (W[:, h-feats]^T src8)  (fp8 DoubleRow)."""
                wp = wk_pan.tile([P, FT, P], F8, tag="wp8")
                nc.sync.dma_start(
                    wp[:],
                    w_dram[:, h * P: (h + 1) * P].rearrange("(dt p) f -> p dt f", p=P),
                )
                for ic in range(2):
                    ps = psum_pr.tile([P, 512], F32, tag="psA")
                    for d in range(FT // 2):
                        nc.tensor.matmul(
                            ps[:],
                            wp[:, 2 * d: 2 * d + 2, :],
                            src8[:, 2 * d: 2 * d + 2, ic * 512: (ic + 1) * 512],
                            start=(d == 0), stop=(d == FT // 2 - 1),
                            perf_mode=DR,
                        )
                    out = dst[:, h, ic * 512: (ic + 1) * 512]
                    if general:
                        nc.vector.tensor_scalar(
                            out=out, in0=ps[:],
                            scalar1=bias_sb[bias_key][:, h: h + 1],
                            scalar2=None, op0=mybir.AluOpType.add,
                        )
                    else:
                        nc.vector.tensor_copy(out, ps[:])

            def qproj(h, ic):
                """qzz[:, h, ic-half], qtok = A @ Wq[:, h-feats] (f16) for
                one query half (panel re-fetched per half)."""
                wp = wq_pan.tile([P, FT, P], F16, tag="wp16")
                nc.sync.dma_start(
                    wp[:],
                    dram["wq"][:, h * P: (h + 1) * P].rearrange(
                        "(dt p) f -> p dt f", p=P
                    ),
                )
                if True:
                    ps = psum_pr.tile([P, 512], F32, tag="psA")
                    for d in range(FT):
                        nc.tensor.matmul(
                            ps[:],
                            wp[:, d, :],
                            at[:, d, ic * 512: (ic + 1) * 512],
                            start=(d == 0), stop=(d == FT - 1),
                        )
                    q16 = q16_pan.tile([P, 512], F16, tag="q16")
                    if general:
                        nc.vector.tensor_scalar(
                            out=q16[:], in0=ps[:],
                            scalar1=bias_sb["bq"][:, h: h + 1],
                            scalar2=None, op0=mybir.AluOpType.add,
                        )
                    else:
                        nc.vector.tensor_copy(q16[:], ps[:])
                    nc.gpsimd.tensor_copy(qzz[:, h, ic * 512: (ic + 1) * 512], q16[:])
                    nc.sync.dma_start_transpose(
                        qtok[:, ic * 4: (ic + 1) * 4, h * P: (h + 1) * P], q16[:]
                    )

            def vproj(h, w_dram, src8, dst, bias_key):
                """dst[:, :, h, 0:DH] = src8^T @ W[:, h-feats]  (fp8 DR)."""
                wp = wv_pan.tile([P, FT, P], F8, tag="wv8")
                nc.sync.dma_start(
                    wp[:],
                    w_dram[:, h * P: (h + 1) * P].rearrange("(dt p) f -> p dt f", p=P),
                )
                for jh in range(2):
                    ps0 = psum_pr.tile([P, 512], F32, tag="psA")
                    ps = ps0[:].rearrange("p (j f) -> p j f", f=P)
                    for j in range(4):
                        for d in range(FT // 2):
                            nc.tensor.matmul(
                                ps[:, j, :],
                                src8[:, 2 * d: 2 * d + 2,
                                     (4 * jh + j) * P: (4 * jh + j + 1) * P],
                                wp[:, 2 * d: 2 * d + 2, :],
                                start=(d == 0), stop=(d == FT // 2 - 1),
                                perf_mode=DR,
                            )
                    out = dst[:, 4 * jh: 4 * jh + 4, h, 0:DH]
                    if general:
                        bc = bias_sb[bias_key][:, h * DH: (h + 1) * DH]
                        bc4 = bass.AP(tensor=bc.tensor, offset=bc.offset,
                                      ap=[bc.ap[0], [0, 4], bc.ap[-1]])
                        nc.vector.tensor_add(out, ps, bc4)
                    else:
                        nc.vector.tensor_copy(out, ps)

            def attn_qk(key, kt, ic, h):
                """QK + exp for one head/half; returns the P tiles."""
                base = qzz[:, h, ic * 512: (ic + 1) * 512]
                qmov = _pair(base, (FT - h) * N)
                p_t = []
                for jp in range(4):
                    sps = psum_s.tile([P, 2, 512], F32, tag="sps")
                    for half in range(2):
                        jt = 2 * jp + half
                        nc.tensor.matmul(
                            sps[:, half, :],
                            _pair(kt[:, h, jt * P: (jt + 1) * P], 0),
                            qmov,
                            start=True, stop=True,
                            perf_mode=DR,
                        )
                    pt = p_pool.tile([P, 2, 512], F8, tag="p_t", name="p_t")
                    nc.scalar.activation(
                        out=pt[:], in_=sps[:],
                        func=mybir.ActivationFunctionType.Exp,
                        scale=SCALE,
                    )
                    p_t.append(pt)
                return p_t

            def attn_pv(key, v_t, ic, h, p_t):
                """PV + denominator + residual z-writes for one head/half."""
                for ip in range(2):
                    pvp = psum_pv.tile([P, 2, DH + 1], F32, tag="pvp")
                    for half in range(2):
                        il = 2 * ip + half
                        for jp in range(4):
                            nc.tensor.matmul(
                                pvp[:, half, :],
                                p_t[jp][:, :, il * P: (il + 1) * P],
                                v_t[:, 2 * jp: 2 * jp + 2, h, :],
                                start=(jp == 0), stop=(jp == 3),
                                perf_mode=DR,
                            )
                    rcp = small.tile([P, 2], F32, tag="rcp")
                    nc.vector.reciprocal(rcp[:], pvp[:, :, DH])
                    for half in range(2):
                        il = 2 * ip + half
                        it = ic * 4 + il
                        nc.vector.scalar_tensor_tensor(
                            out=zc[key][:, it, h * DH: (h + 1) * DH],
                            in0=pvp[:, half, 0:DH],
                            scalar=rcp[:, half: half + 1],
                            in1=qtok[:, it, h * DH: (h + 1) * DH],
                            op0=mybir.AluOpType.mult,
                            op1=mybir.AluOpType.add,
                        )

            def emit_ln(key, it):
                """LN zc[key][:, it, :] -> lt[key] feature-major (transpose)."""
                mean, rstd = ln_rstd(zc[key][:, it, :])
                ltok = ltok_pool.tile([P, D], F16, tag="ltok")
                nc.gpsimd.tensor_scalar(
                    out=ltok[:], in0=zc[key][:, it, :],
                    scalar1=mean, scalar2=rstd,
                    op0=mybir.AluOpType.subtract, op1=mybir.AluOpType.mult,
                )
                nc.sync.dma_start_transpose(
                    lt[key][:, :, it * P: (it + 1) * P], ltok[:]
                )

            # ---------------- block 1: per-h K1,Q(ic0),V1,QK; PV deferred ---
            pend = None  # (key, v_t, ic, h, p_t)
            for h in range(H):
                kproj(h, dram["wk1"], at8, k1t, "bk1")
                if h == 0:
                    nc.sync.dma_start(
                        at[:], dram["at"][:].rearrange("(t p) n -> p t n", p=P)
                    )
                qproj(h, 0)
                vproj(h, dram["wv1"], at8, v1, "bv1")
                p_t = attn_qk("own", k1t, 0, h)
                if pend is not None:
                    attn_pv(*pend)
                pend = ("own", v1, 0, h, p_t)

            # at8 -> ct8 space swap; FC weights stream on SWDGE
            pool_a8.release()
            pool_c = tc.alloc_tile_pool(name="pool_c", bufs=1, side="right")
            ct8 = pool_c.tile([P, FT, N], F8, tag="ct8", name="ct8_all")
            nc.gpsimd.dma_start(ct8[:], dram["ct8"][:].rearrange("(t p) n -> p t n", p=P))
            for oc in range(2):
                for nm in ("w1", "w2"):
                    nc.gpsimd.dma_start(
                        wps[(nm, oc)][:],
                        dram[nm][:, oc * 512: (oc + 1) * 512].rearrange(
                            "(dt p) f -> p dt f", p=P
                        ),
                    )

            # ---------------- block 2: per-h K2,V2,Q(ic1),QK; PV deferred ---
            for h in range(H):
                kproj(h, dram["wk2"], ct8, k2t, "bk2")
                vproj(h, dram["wv2"], ct8, v2, "bv2")
                qproj(h, 1)
                p_t = attn_qk("oth", k2t, 0, h)
                attn_pv(*pend)
                pend = ("oth", v2, 0, h, p_t)
            attn_pv(*pend)
            pend = None
            pool_c.release()
            pool_at.release()

            # proj psum banks -> FC psum
            psum_pr.release()
            psum_fc = tc.alloc_tile_pool(name="psum_fc", bufs=2, space="PSUM")
            fc_pools = [psum_fc]

            # LN token tiles 0-3 (both attns) while block 3 runs
            for it in range(4):
                emit_ln("own", it)
                emit_ln("oth", it)

            # --------- FC machinery: 16 matmuls per (it, oc) group ----------
            fc_state = {}

            def fc_half(idx):
                """Half-group idx: 8 of the 16 matmuls of group g = idx//2."""
                g = idx // 2
                it, oc = g // 2, g % 2
                key, wnm = (("own", "w1") if idx % 2 == 0 else ("oth", "w2"))
                if idx % 2 == 0:
                    fc_state[g] = fc_pools[-1].tile(
                        [P, 512], F32, tag="fps", name="fps"
                    )
                    if oc == 0:
                        fc_state[("u", it)] = u_pool.tile(
                            [P, D], F16, tag="ut", name="ut"
                        )
                fps = fc_state[g]
                for kt_i in range(FT):
                    nc.tensor.matmul(
                        fps[:],
                        lt[key][:, kt_i, it * P: (it + 1) * P],
                        wps[(wnm, oc)][:, kt_i, :],
                        start=(idx % 2 == 0 and kt_i == 0),
                        stop=(idx % 2 == 1 and kt_i == FT - 1),
                    )
                if idx % 2 == 1:
                    ut = fc_state[("u", it)]
                    if general:
                        nc.vector.scalar_tensor_tensor(
                            out=ut[:, oc * 512: (oc + 1) * 512],
                            in0=fps[:], scalar=1.0,
                            in1=bias_sb["fcb"][:, oc * 512: (oc + 1) * 512],
                            op0=mybir.AluOpType.mult, op1=mybir.AluOpType.add,
                        )
                    else:
                        nc.vector.tensor_scalar(
                            out=ut[:, oc * 512: (oc + 1) * 512],
                            in0=fps[:], scalar1=0.0, scalar2=None,
                            op0=mybir.AluOpType.max,
                        )
                        # u-LN stats for this half right away (hides the
                        # bn_stats latency behind the other oc's matmuls)
                        if oc == 0:
                            fc_state[("st", it)] = small.tile(
                                [P, 2, 6], F32, tag="u_stats", name="ust"
                            )
                        nc.vector.bn_stats(
                            out=fc_state[("st", it)][:, oc, :],
                            in_=ut[:, oc * 512: (oc + 1) * 512],
                        )
                    if oc == 1:
                        finish_token(it)

            def finish_token(it):
                """relu'd ut complete: final LN + store."""
                ut = fc_state.pop(("u", it))
                if general:
                    nc.gpsimd.tensor_scalar(
                        out=ut[:], in0=ut[:], scalar1=0.0, scalar2=None,
                        op0=mybir.AluOpType.max,
                    )
                    mean, rstd = ln_rstd(ut[:])
                else:
                    mean, rstd = ln_aggr(fc_state.pop(("st", it)))
                ot = out_pool.tile([P, D], F32, tag="ot")
                nc.gpsimd.tensor_scalar(
                    out=ot[:], in0=ut[:], scalar1=mean, scalar2=rstd,
                    op0=mybir.AluOpType.subtract, op1=mybir.AluOpType.mult,
                )
                if general:
                    nc.vector.tensor_mul(ot[:], ot[:], bias_sb["g1v"][:])
                    nc.vector.tensor_add(ot[:], ot[:], bias_sb["b1v"][:])
                nc.sync.dma_start(o_dram[it * P: (it + 1) * P, :], ot[:])

            # ---------------- block 3: own attn ic1 + FC halves 0-5 ---------
            for h in range(H):
                p_t = attn_qk("own", k1t, 1, h)
                if pend is not None:
                    attn_pv(*pend)
                if h >= 2:
                    fc_half(h - 2)
                pend = ("own", v1, 1, h, p_t)

            # ------- block 4: oth attn ic1 + FC 6-15 + LN(own,4-7) ----------
            for h in range(H):
                p_t = attn_qk("oth", k2t, 1, h)
                attn_pv(*pend)
                if h < 6:
                    fc_half(6 + h)
                else:
                    fc_half(2 * h)       # h6 -> 12, h7 -> 14
                    fc_half(2 * h + 1)   # h6 -> 13, h7 -> 15
                if h < 4:
                    emit_ln("own", 4 + h)  # token tiles 4..7 (zc own done)
                pend = ("oth", v2, 1, h, p_t)
            attn_pv(*pend)

            # ---------------- tail: its 4-7, own-halves lead ----------------
            psum_fc.release()
            psum_pv.release()
            fc_pools.append(
                tc.alloc_tile_pool(name="psum_fc2", bufs=4, space="PSUM")
            )
            emit_ln("oth", 4)
            emit_ln("oth", 5)
            fc_half(16); fc_half(18); fc_half(17); fc_half(19)
            emit_ln("oth", 6)
            fc_half(20); fc_half(22); fc_half(21); fc_half(23)
            emit_ln("oth", 7)
            fc_half(24); fc_half(26); fc_half(25); fc_half(27)
            fc_half(28); fc_half(30); fc_half(29); fc_half(31)

            fc_pools[-1].release()
            psum_s.release()

    nc.compile()
    return nc


def build_in_maps(X, Y, Wqx, bqx, Wkx, bkx, Wvx, bvx, Wqy, bqy, Wky, bky,
                  Wvy, bvy, WX, bX, WY, bY, g0, b0, g1, b1):
    f = lambda t: np.asarray(t, dtype=np.float32)
    h16 = lambda t: np.ascontiguousarray(f(t).astype(np.float16))
    h8 = lambda t: np.ascontiguousarray(f(t).astype(np.float16).astype(NF8))
    X, Y = f(X), f(Y)
    g0d, b0d = f(g0).astype(np.float64), f(b0).astype(np.float64)

    general = not (
        all(np.allclose(f(b), 0.0) for b in (bqx, bkx, bvx, bqy, bky, bvy,
                                             bX, bY, b0, b1))
        and np.allclose(f(g1), 1.0)
    )

    sides = {}
    for side, W, bo in (("x", f(WX), f(bX)), ("y", f(WY), f(bY))):
        Wtop = W[:D].astype(np.float64)
        Wbot = W[D:].astype(np.float64)
        fcb = (b0d @ Wtop + b0d @ Wbot + bo.astype(np.float64)).astype(np.float32)
        w_top_folded = (g0d[:, None] * Wtop).astype(np.float32)
        w_bot_folded = (g0d[:, None] * Wbot).astype(np.float32)
        if side == "x":
            w_own, w_oth = w_top_folded, w_bot_folded  # [O_xx, O_xy]
        else:
            w_own, w_oth = w_bot_folded, w_top_folded  # [O_yx, O_yy]
        sides[side] = dict(w1=h16(w_own), w2=h16(w_oth), fcb=fcb)

    wx = dict(wq=h16(Wqx), bq=f(bqx), wk=h8(Wkx), bk=f(bkx), wv=h8(Wvx),
              bv=f(bvx))
    wy = dict(wq=h16(Wqy), bq=f(bqy), wk=h8(Wky), bk=f(bky), wv=h8(Wvy),
              bv=f(bvy))

    seq_t = {}
    for nm, S in (("x", X), ("y", Y)):
        for b in range(S.shape[0]):
            t16 = np.ascontiguousarray(S[b].T.astype(np.float16))
            seq_t[(nm, b)] = (t16, np.ascontiguousarray(t16.astype(NF8)))

    in_maps = []
    for core in range(8):
        b = core // 2
        side = "x" if core % 2 == 0 else "y"
        own, oth = (wx, wy) if side == "x" else (wy, wx)
        a_t = seq_t[(side, b)]
        c_t = seq_t[("y" if side == "x" else "x", b)]
        m = {
            "at": a_t[0], "at8": a_t[1], "ct8": c_t[1],
            "wq": own["wq"],
            "wk1": own["wk"], "wv1": own["wv"],
            "wk2": oth["wk"], "wv2": oth["wv"],
            "w1": sides[side]["w1"], "w2": sides[side]["w2"],
        }
        if general:
            m.update({
                "bq": own["bq"], "bk1": own["bk"], "bv1": own["bv"],
                "bk2": oth["bk"], "bv2": oth["bv"],
                "fcb": sides[side]["fcb"], "g1v": f(g1), "b1v": f(b1),
            })
        in_maps.append(m)
    return in_maps, general


def kernel(**inputs):
    import time as _time

    in_maps, general = build_in_maps(**inputs)
    key = ("nc", general)
    if key not in _CACHED:
        _CACHED[key] = _build(general)
    nc = _CACHED[key]
    _CACHED["nc"] = nc  # test.py compatibility

    res = None
    for attempt in range(4):
        try:
            res = run_bass_kernel_spmd(nc, in_maps, list(range(8)))
            break
        except Exception:
            if attempt == 3:
                raise
            _time.sleep(2.0)
    _CACHED["last_result"] = res

    B = np.asarray(inputs["X"]).shape[0]
    O_x = np.stack([res.results[2 * b]["o"] for b in range(B)])
    O_y = np.stack([res.results[2 * b + 1]["o"] for b in range(B)])
    return O_x, O_y
